# revision 31
# baseline (speedup 1.0000x reference)
"""Trainium2 Bass kernel for nn_AudioSelfAttention (B=2, T=2048, C=1024, H=16).

Sharding: sequence-parallel over the 8 NeuronCores. Core i handles batch
i//4 and query-token slice (i%4)*512. Each core computes K/V for its full
batch locally (redundant within the 4-core batch group — measured collective
cost here, ~76us floor + ~47us/MB, makes the AllGather alternative slower
than recomputation), computes attention for its own 512 query rows over all
16 heads, and the output projection. No collectives; the full output is
assembled on the host from the 8 row-shards.

The per-core xt input is ROLLED on the host so the core's own query block is
token-chunk 0 (attention is permutation-invariant over keys): q is computed
straight from xt chunk 0 and no separate xq input is needed.

Engine balance: ScalarE's exp stream (~1.25us per kt-chunk) is the pair-phase
floor, so pair 0's attention chunks are processed INLINE with the K0/K1/V
streaming phase (PE-heavy, exp fully hidden), and pairs 1-7 each drip-feed
the NEXT pair's K projection (2 matmuls per chunk) plus a spread schedule of
proj-partial groups, sized so every pair stays just under the exp pace.
xt is DMA'd in token chunks (chunk 0 feature-chunk-wise) so the Q phase
starts after ~130KB. The sync DMA queue carries only small weight blocks — a
descriptor-ring-full stall on the sync engine blocks tile-pool boundaries
and thus the PE — and all inputs are pre-shuffled on the host to
partition-major layouts so every DMA trigger generates only ~128 descriptors.

Compute dtype: bf16 matmul operands (fp32 matmul is 4 cycles/row on TRN2's
PE vs 1 for bf16), fp32 PSUM accumulation and softmax statistics. Output is
stored bf16 (quantization ~0.4% rel, well within budget) and upcast on host.

Layouts: activations live in transposed [feature, token] space so every
matmul contracts along partitions. Attention scores are computed as
S^T = K^T-chunks x Q^T (kt on partitions), softmax-exp runs on ScalarE over
4-bank PSUM spans, and the softmax denominator falls out of the y-matmul by
augmenting V with a ones-column (M=65). Per-pair normalization uses two tiny
half-broadcast matmuls (ones-row stationary) to spread 1/sum across
partitions. The v-part and proj biases are folded in exactly on the host
(softmax rows sum to 1, so they reduce to a constant row added to the
output); q/k biases are applied on-device in the PSUM->SBUF copies.
All proj contributions except the last feature chunk are accumulated during
the final two head-pairs, so the post-attention tail is just the last-pair
normalization, 8 single matmuls, and the (bf16) output DMA.
"""
import numpy as np

_CACHE = {}

B, T, C, H, D = 2, 2048, 1024, 16, 64
TQ = T * B // 8          # 512 query tokens per core
CC = C // 128            # 8 contraction chunks
NPAIR = H // 2           # 8 head pairs
NKT = T // 128           # 16 kt chunks


def _build_nc():
    import concourse.bacc as bacc
    import concourse.tile as tile
    import concourse.mybir as mybir

    f32 = mybir.dt.float32
    bf16 = mybir.dt.bfloat16
    Exp = mybir.ActivationFunctionType.Exp

    nc = bacc.Bacc(None, num_devices=8)
    # All inputs are pre-shuffled on the host into partition-major layouts so
    # every DMA is ~128 contiguous per-partition segments (cheap descriptor
    # generation — a (c p)-rearranged DMA costs 1024 descriptors and 8-28us
    # of trigger time on the issuing engine).
    # xt[p, cc, t]: rolled so this core's query block is token chunk 0
    xt = nc.declare_dram_parameter("xt", [128, CC, T], bf16, isOutput=False)
    # wqk[jc, p, cc, j] = W_attn[cc*128+p, jc*128+j]  (jc 0..7 = q, 8..15 = k)
    wqk = nc.declare_dram_parameter("wqk", [16, 128, CC, 128], bf16, isOutput=False)
    wv_in = nc.declare_dram_parameter("wv_in", [128, 2, CC, TQ], bf16, isOutput=False)
    ba = nc.declare_dram_parameter("ba", [128, 16], f32, isOutput=False)
    wp = nc.declare_dram_parameter("wp", [128, CC, C], bf16, isOutput=False)
    out = nc.declare_dram_parameter("out", [TQ, C], bf16, isOutput=True)

    with tile.TileContext(nc) as tc:
        with (
            tc.tile_pool(name="big", bufs=1) as big,
            tc.tile_pool(name="wst", bufs=3) as wst,
            tc.tile_pool(name="kpool", bufs=4) as kpool,
            tc.tile_pool(name="pexp", bufs=6) as pexp,
            tc.tile_pool(name="small", bufs=2) as small,
            tc.tile_pool(name="mmps", bufs=2, space="PSUM") as mmps,
            tc.tile_pool(name="spool", bufs=2, space="PSUM") as spool,
            tc.tile_pool(name="ypool", bufs=1, space="PSUM") as ypool,
        ):
            # ---- DMA schedule.
            # scalar queue: xt chunk 0 (cc-wise, so the Q phase starts after
            #   ~130KB), wv in dh halves, wp.
            # sync queue: wq blocks + ba, wk01, xt token chunks 1-3 (cc-wise
            #   contiguous tails), then per-pair wk drip later. ----
            xt_sb = big.tile([128, CC, T], bf16)
            for cc in range(CC):
                nc.scalar.dma_start(xt_sb[:, cc, 0:TQ], xt[:, cc, 0:TQ])

            # wq shares its slot with the proj partial accumulator (disjoint lifetimes)
            wq_all = big.tile([128, CC, CC, 128], bf16, tag="scratch16")
            for jc in range(2):
                nc.sync.dma_start(wq_all[:, jc, :, :], wqk[jc])
            ba_sb = big.tile([128, 16], f32)
            nc.sync.dma_start(ba_sb[:], ba[:])
            wk0 = wst.tile([128, CC, 128], bf16, tag="w", name="wk_0")
            nc.sync.dma_start(wk0[:], wqk[8])
            for jc in range(2, CC):
                nc.sync.dma_start(wq_all[:, jc, :, :], wqk[jc])
            wk1 = wst.tile([128, CC, 128], bf16, tag="w", name="wk_1")
            nc.sync.dma_start(wk1[:], wqk[9])

            # bulk tails ride the scalar queue — the sync queue stays lean so
            # its engine never hits a ring-full stall (tile-pool boundaries
            # ride the sync engine and a stalled trigger there blocks the PE)
            wv_sb = big.tile([128, 2, CC, TQ], bf16)
            nc.scalar.dma_start(wv_sb[:, 0], wv_in[:, 0])
            for cc in range(CC):
                nc.scalar.dma_start(xt_sb[:, cc, TQ:T], xt[:, cc, TQ:T])
            # dh1 half of wv is consumed only by the V drip in pairs 1-3
            nc.scalar.dma_start(wv_sb[:, 1], wv_in[:, 1])
            wp_sb = big.tile([128, CC, C], bf16)
            nc.scalar.dma_start(wp_sb[:], wp[:])

            sel_sb = big.tile([1, 64], bf16)
            nc.vector.memset(sel_sb[:], 1.0)
            q_sb = big.tile([128, CC, TQ], bf16)
            # v pair blocks padded to 144 cols (288B, 16B-aligned for both
            # head slices): [vA(64) | 1 | pad(7) | vB(64) | 1 | pad(7)].
            # One extra pad block lets M=128 stationary reads over-run (junk
            # cols only land in unread PSUM partitions 65..127).
            v_sb = big.tile([128, NKT, NPAIR + 1, 144], bf16)
            nc.vector.memset(
                v_sb.rearrange("p t r (h f) -> p t r h f", h=2)[:, :, :, :, 64:65], 1.0
            )
            yt_sb = big.tile([128, CC, TQ], bf16)
            yhat_sb = big.tile([128, CC, TQ], bf16)

            # ---- PE warm-up: the PE needs ~3us of continuous work to reach
            # the 2.4GHz p-state; burn the DMA lead-in on dummy matmuls over
            # the first xt chunk (output discarded) ----
            warm_ps = mmps.tile([128, TQ], f32, tag="mm", name="warm_ps")
            for w in range(8):
                nc.tensor.matmul(warm_ps[:], xt_sb[:, 0, 0:128],
                                 xt_sb[:, 0, 0:TQ], start=True, stop=True)

            # ---- Q phase: q^T[j, tq] = W_q^T @ xt[:, chunk0], cc-outer over
            # jc pairs so the first matmul needs only wq[0,1] + xt chunk0/cc0.
            # PSUM alternates mmps/ypool so groups don't stall on rotation. ----
            for jcg in range(CC // 2):
                jA, jB = 2 * jcg, 2 * jcg + 1
                if jcg % 2 == 0:
                    qA = mmps.tile([128, TQ], f32, tag="mm", name=f"qps_{jA}")
                    qB = mmps.tile([128, TQ], f32, tag="mm", name=f"qps_{jB}")
                else:
                    qAB = ypool.tile([128, 2 * TQ], f32, tag="y", name=f"qps_{jA}{jB}")
                    qA, qB = qAB[:, 0:TQ], qAB[:, TQ:2 * TQ]
                for cc in range(CC):
                    nc.tensor.matmul(qA[:], wq_all[:, jA, cc, :], xt_sb[:, cc, 0:TQ],
                                     start=(cc == 0), stop=(cc == CC - 1))
                    nc.tensor.matmul(qB[:], wq_all[:, jB, cc, :], xt_sb[:, cc, 0:TQ],
                                     start=(cc == 0), stop=(cc == CC - 1))
                nc.vector.tensor_scalar_add(q_sb[:, jA, :], qA[:], ba_sb[:, jA:jA + 1])
                nc.vector.tensor_scalar_add(q_sb[:, jB, :], qB[:], ba_sb[:, jB:jB + 1])

            # deferred per-pair normalization: emitted 3 chunks into the NEXT
            # pair so the sums->reciprocal chain never stalls the in-order PE
            # at a pair boundary
            s2_tiles = {}

            def emit_norm(p):
                s2 = s2_tiles.pop(p)
                r2 = small.tile([1, 2 * TQ], f32, tag="r2", name=f"r2_{p}", bufs=1)
                nc.vector.reciprocal_approx_fast(r2[:], s2[:])
                rb2 = small.tile([1, 2 * TQ], bf16, tag="rb2", name=f"rb2_{p}", bufs=1)
                nc.vector.tensor_copy(rb2[:], r2[:])
                bc = mmps.tile([128, TQ], f32, tag="mm", name=f"bc_{p}")
                nc.tensor.matmul(bc[0:64, :], sel_sb[:], rb2[:, 0:TQ], start=True,
                                 stop=True)
                nc.tensor.matmul(bc[64:128, :], sel_sb[:], rb2[:, TQ:2 * TQ],
                                 start=True, stop=True)
                nc.vector.tensor_mul(yt_sb[0:64, p, :], yhat_sb[0:64, p, :], bc[0:64, :])
                nc.vector.tensor_mul(yt_sb[64:128, p, :], yhat_sb[64:128, p, :],
                                     bc[64:128, :])

            def emit_drain(p, yAB):
                yA, yB = yAB[:, 0:TQ], yAB[:, TQ:2 * TQ]
                s2 = small.tile([1, 2 * TQ], f32, tag="s2", name=f"s2_{p}")
                nc.vector.tensor_copy(s2[:], yAB[64:65, :])
                if p == NPAIR - 1:
                    # last pair: sums first, bc matmuls overlap the yhat
                    # copies — shortest tail chain
                    r2 = small.tile([1, 2 * TQ], f32, tag="r2", name=f"r2_{p}", bufs=1)
                    nc.vector.reciprocal_approx_fast(r2[:], s2[:])
                    rb2 = small.tile([1, 2 * TQ], bf16, tag="rb2", name=f"rb2_{p}",
                                     bufs=1)
                    nc.vector.tensor_copy(rb2[:], r2[:])
                    bc = mmps.tile([128, TQ], f32, tag="mm", name=f"bc_{p}")
                    nc.tensor.matmul(bc[0:64, :], sel_sb[:], rb2[:, 0:TQ], start=True,
                                     stop=True)
                    nc.tensor.matmul(bc[64:128, :], sel_sb[:], rb2[:, TQ:2 * TQ],
                                     start=True, stop=True)
                    nc.vector.tensor_copy(yhat_sb[0:64, p, :], yA[0:64, :])
                    nc.vector.tensor_copy(yhat_sb[64:128, p, :], yB[0:64, :])
                    nc.vector.tensor_mul(yt_sb[0:64, p, :], yhat_sb[0:64, p, :],
                                         bc[0:64, :])
                    nc.vector.tensor_mul(yt_sb[64:128, p, :], yhat_sb[64:128, p, :],
                                         bc[64:128, :])
                else:
                    # stash unnormalized y + sums, freeing the accumulator
                    s2_tiles[p] = s2
                    nc.vector.tensor_copy(yhat_sb[0:64, p, :], yA[0:64, :])
                    nc.vector.tensor_copy(yhat_sb[64:128, p, :], yB[0:64, :])

            def s_chunk(p, c, k_t, q_col, pe_tiles, split_exp=False):
                tt, off = c // 4, (c % 4) * 128
                sp = spool.tile([128, 2 * TQ], f32, tag="s", name=f"s_{p}_{c}")
                nc.tensor.matmul(sp[:, 0:TQ], k_t[0:64, tt, off:off + 128],
                                 q_sb[0:64, q_col, :], start=True, stop=True)
                nc.tensor.matmul(sp[:, TQ:2 * TQ], k_t[64:128, tt, off:off + 128],
                                 q_sb[64:128, q_col, :], start=True, stop=True)
                pe_t = pexp.tile([128, 2 * TQ], bf16, tag="pe", name=f"pe_{p}_{c}")
                if split_exp:
                    # y(c,A) can start while the B half is still on ScalarE
                    nc.scalar.activation(pe_t[:, 0:TQ], sp[:, 0:TQ], Exp, scale=0.125)
                    nc.scalar.activation(pe_t[:, TQ:2 * TQ], sp[:, TQ:2 * TQ],
                                         Exp, scale=0.125)
                else:
                    nc.scalar.activation(pe_t[:], sp[:], Exp, scale=0.125)
                pe_tiles[c] = pe_t

            def make_emit_y(p, yA, yB, pe_tiles):
                def emit_y(c):
                    pe_t = pe_tiles.pop(c)
                    vflat = v_sb[:, c].rearrange("p r f -> p (r f)")
                    nc.tensor.matmul(yA[:], vflat[:, p * 144:p * 144 + 128],
                                     pe_t[:, 0:TQ], start=(c == 0), stop=(c == NKT - 1))
                    nc.tensor.matmul(yB[:], vflat[:, p * 144 + 72:p * 144 + 200],
                                     pe_t[:, TQ:2 * TQ],
                                     start=(c == 0), stop=(c == NKT - 1))
                return emit_y


            def emit_v(ci, dh):
                v_ps = mmps.tile([128, TQ], f32, tag="mm", name=f"vps_{ci}_{dh}")
                for cc in range(CC):
                    nc.tensor.matmul(v_ps[:], xt_sb[:, cc, ci * 128:(ci + 1) * 128],
                                     wv_sb[:, dh, cc, :],
                                     start=(cc == 0), stop=(cc == CC - 1))
                nc.vector.tensor_copy(
                    v_sb[:, ci, 4 * dh:4 * dh + 4, :]
                    .rearrange("p r (h f) -> p r h f", h=2)[:, :, :, 0:64],
                    v_ps.rearrange("p (r h f) -> p r h f", r=4, h=2),
                )

            # ---- phase 1b: stream K0/K1 + the dh0 half of V per token chunk,
            # processing pair 0's attention chunks inline — ScalarE's exp
            # stream hides under the V matmuls. ----
            kt0 = kpool.tile([128, 4, TQ], bf16, tag="kp", name="k_0")
            kt1 = kpool.tile([128, 4, TQ], bf16, tag="kp", name="k_1")
            yAB0 = ypool.tile([128, 2 * TQ], f32, tag="y", name="yAB_0")
            pe0 = {}
            emit_y0 = make_emit_y(0, yAB0[:, 0:TQ], yAB0[:, TQ:2 * TQ], pe0)
            for tt in range(4):
                for j, (wk_t, k_t) in enumerate(((wk0, kt0), (wk1, kt1))):
                    k_ps = mmps.tile([128, TQ], f32, tag="mm", name=f"kps_{j}_{tt}")
                    for cc in range(CC):
                        nc.tensor.matmul(k_ps[:], wk_t[:, cc, :],
                                         xt_sb[:, cc, tt * TQ:(tt + 1) * TQ],
                                         start=(cc == 0), stop=(cc == CC - 1))
                    nc.vector.tensor_scalar_add(k_t[:, tt, :], k_ps[:],
                                                ba_sb[:, 8 + j:9 + j])
                for ci in range(4 * tt, 4 * tt + 4):
                    emit_v(ci, 0)
                    s_chunk(0, ci, kt0, 0, pe0)
                    if ci >= 2:
                        emit_y0(ci - 2)
            emit_y0(NKT - 2)
            emit_y0(NKT - 1)
            emit_drain(0, yAB0)
            ktiles = {1: kt1}

            # proj partial accumulator (slot shared with wq_all, whose
            # lifetime ends after the Q phase)
            opart_sb = big.tile([128, CC, TQ], f32, tag="scratch16")

            # V-dh1 drip: pairs 4-7's V half is computed during the exp-paced
            # pairs 1-3 (one transient psum block per kn-ring window; pair 1
            # fits two). {pair: {c_slot: v_chunk}}
            _VDH1_SCHED = {
                1: {1: 0, 3: 1, 5: 2, 7: 3, 9: 4, 11: 5, 13: 6, 15: 7},
                2: {3: 8, 7: 9, 11: 10, 15: 11},
                3: {3: 12, 7: 13, 11: 14, 15: 15},
            }

            # proj-partial drip schedule: pair -> {c_slot: [(tile, cc_lo, single)]}
            # 3 groups per pair at c={5,9,13} for pairs 2-6 (inside the kn-drip
            # ring pattern), remainder in pair 7 which has no K drip.
            _PROJ_SCHED = {
                2: {5: [(0, 0, False)], 9: [(1, 0, False)], 13: [(2, 0, False)]},
                3: {5: [(3, 0, False)], 9: [(4, 0, False)], 13: [(5, 0, False)]},
                4: {5: [(6, 0, False)], 9: [(7, 0, False)], 13: [(0, 2, False)]},
                5: {5: [(1, 2, False)], 9: [(2, 2, False)], 13: [(3, 2, False)]},
                6: {5: [(4, 2, False)], 9: [(5, 2, False)], 13: [(6, 2, False)]},
                7: {3: [(7, 2, False)], 4: [(0, 4, False)], 5: [(1, 4, False)],
                    6: [(2, 4, False)], 7: [(3, 4, False)], 8: [(4, 4, False)],
                    9: [(5, 4, False)], 10: [(6, 4, False)], 11: [(7, 4, False)],
                    12: [(0, 6, True), (1, 6, True)],
                    13: [(2, 6, True), (3, 6, True)],
                    14: [(4, 6, True), (5, 6, True)],
                    15: [(6, 6, True), (7, 6, True)]},
            }

            # ---- pairs 1-7 ----
            # Pipeline per pair: per kt-chunk c emit S(c) -> exp(c) -> y(c-1),
            # with the next pair's K-chunk matmuls drip-fed 2 per chunk so the
            # PE always has exp-independent work while ScalarE runs.
            for p in range(1, NPAIR):
                kt_next = p + 1 if p + 1 < NPAIR else None
                knext_state = {}

                def emit_knext(ci, p=p, kt_next=kt_next, st=None):
                    # two accumulation matmuls of k(p+1) per kt chunk index ci
                    if kt_next is None:
                        return
                    st = knext_state
                    tt, ai = ci // 4, (ci % 4) * 2
                    if ai == 0:
                        st["wk"] = wst.tile([128, CC, 128], bf16, tag="w",
                                            name=f"wkn_{kt_next}_{tt}") if tt == 0 else st["wk"]
                        if tt == 0:
                            nc.sync.dma_start(st["wk"][:], wqk[8 + kt_next])
                            st["kt"] = kpool.tile([128, 4, TQ], bf16, tag="kp",
                                                  name=f"k_{kt_next}")
                        st["ps"] = mmps.tile([128, TQ], f32, tag="mm",
                                             name=f"kn_{kt_next}_{tt}")
                    for cc in (ai, ai + 1):
                        nc.tensor.matmul(st["ps"][:], st["wk"][:, cc, :],
                                         xt_sb[:, cc, tt * TQ:(tt + 1) * TQ],
                                         start=(cc == 0), stop=(cc == CC - 1))
                    if ai == 6:
                        nc.vector.tensor_scalar_add(st["kt"][:, tt, :], st["ps"][:],
                                                    ba_sb[:, 8 + kt_next:9 + kt_next])
                        if tt == 3:
                            ktiles[kt_next] = st["kt"]

                k_t = ktiles.pop(p)
                yAB = ypool.tile([128, 2 * TQ], f32, tag="y", name=f"yAB_{p}")
                pe_tiles = {}
                emit_y = make_emit_y(p, yAB[:, 0:TQ], yAB[:, TQ:2 * TQ], pe_tiles)

                # proj partials spread over pairs 2-7, sized so every pair
                # stays under the exp pace (~170ns/chunk of free PE): 2-cc
                # groups accumulated into opart as the yt columns become
                # available (yt[cc] needs norm(cc), done at pair cc+1 c==2).
                # PROJ_SCHED[p] = [(c_slot, tile, cc_lo or cc6-single)].
                def emit_projpart(c, p=p):
                    sched = _PROJ_SCHED.get(p)
                    if not sched or c not in sched:
                        return
                    for gi, (i, cc_lo, single) in enumerate(sched[c]):
                        tt, oh = i // 2, i % 2
                        pp = mmps.tile([128, TQ], f32, tag="mm",
                                       name=f"pp_{p}_{c}_{gi}")
                        nc.tensor.matmul(pp[:],
                                         yt_sb[:, cc_lo, tt * 128:(tt + 1) * 128],
                                         wp_sb[:, cc_lo, oh * TQ:(oh + 1) * TQ],
                                         start=True, stop=single)
                        if not single:
                            nc.tensor.matmul(
                                pp[:], yt_sb[:, cc_lo + 1, tt * 128:(tt + 1) * 128],
                                wp_sb[:, cc_lo + 1, oh * TQ:(oh + 1) * TQ],
                                start=False, stop=True)
                        if cc_lo == 0:
                            nc.vector.tensor_copy(opart_sb[:, i, :], pp[:])
                        else:
                            nc.vector.tensor_add(opart_sb[:, i, :], opart_sb[:, i, :],
                                                 pp[:])

                for c in range(NKT):
                    s_chunk(p, c, k_t, p, pe_tiles,
                            split_exp=(p == NPAIR - 1 and c == NKT - 1))
                    if c >= 2:
                        emit_y(c - 2)
                    emit_knext(c)
                    vci = _VDH1_SCHED.get(p, {}).get(c)
                    if vci is not None:
                        emit_v(vci, 1)
                    emit_projpart(c)
                    if c == (1 if p == NPAIR - 1 else 2):
                        emit_norm(p - 1)
                emit_y(NKT - 2)
                emit_y(NKT - 1)
                emit_drain(p, yAB)

            # ---- output projection: last contraction chunk + stashed partials.
            # Both oh-halves of a token row go into one (now idle) spool tile,
            # so the tail is 4 wide adds and 4 full-row DMAs. ----
            for tt in range(4):
                o_ps = spool.tile([128, 2 * TQ], f32, tag="s", name=f"ops_{tt}")
                for oh in range(2):
                    nc.tensor.matmul(o_ps[:, oh * TQ:(oh + 1) * TQ],
                                     yt_sb[:, CC - 1, tt * 128:(tt + 1) * 128],
                                     wp_sb[:, CC - 1, oh * TQ:(oh + 1) * TQ],
                                     start=True, stop=True)
                o_sb = small.tile([128, 2 * TQ], bf16, tag=f"osb{tt}", name=f"osb_{tt}",
                                  bufs=1)
                nc.vector.tensor_add(
                    o_sb[:], o_ps[:],
                    opart_sb[:, 2 * tt:2 * tt + 2, :].rearrange("p i t -> p (i t)"))
                dma_eng = (nc.sync, nc.scalar, nc.gpsimd)[tt % 3]
                dma_eng.dma_start(out[tt * 128:(tt + 1) * 128, :], o_sb[:])
    nc.compile()
    return nc


def _get_nc():
    if "nc" not in _CACHE:
        _CACHE["nc"] = _build_nc()
    return _CACHE["nc"]


def _in_maps(x, W_attn, b_attn, W_proj, b_proj):
    import ml_dtypes
    bf = ml_dtypes.bfloat16
    x = np.asarray(x, np.float32).reshape(B, T, C)
    W_attn = np.asarray(W_attn, np.float32)
    b_attn = np.asarray(b_attn, np.float32)
    W_proj = np.asarray(W_proj, np.float32)

    xt_all = [x[b_].T.astype(bf) for b_ in range(B)]
    # jc-major contiguous q/k weight blocks: wqk[jc, p, cc, j]
    wqk = np.ascontiguousarray(
        W_attn[:, :2 * C].reshape(CC, 128, 16, 128).transpose(2, 1, 0, 3)
    ).astype(bf)
    # wv[p, dh, cc, t] = W_v[cc*128+p, dh*512+t] — partition-major halves
    wv = np.ascontiguousarray(
        W_attn[:, 2 * C:].reshape(CC, 128, 2, TQ).transpose(1, 2, 0, 3)).astype(bf)
    # wp[p, cc, d] = W_proj[cc*128+p, d]
    wp = np.ascontiguousarray(
        W_proj.reshape(CC, 128, C).transpose(1, 0, 2)).astype(bf)
    # ba pre-shuffled to [128, 16] so the DMA is 128 contiguous 64B rows
    ba = np.ascontiguousarray(b_attn[:2 * C].reshape(16, 128).T.astype(np.float32))

    maps = []
    for i in range(8):
        b_, r = i // 4, i % 4
        xt_b = xt_all[b_]
        if r:
            # roll so this core's query block is token chunk 0 (attention is
            # permutation-invariant over keys)
            xt_b = np.concatenate([xt_b[:, r * TQ:], xt_b[:, :r * TQ]], axis=1)
        # xt[p, cc, t] = xt_b[cc*128+p, t] — partition-major
        xt_b = np.ascontiguousarray(xt_b.reshape(CC, 128, T).transpose(1, 0, 2))
        maps.append({"xt": xt_b, "wqk": wqk, "wv_in": wv, "ba": ba, "wp": wp})
    return maps


def run(x, W_attn, b_attn, W_proj, b_proj, trace=False):
    from concourse.bass_utils import run_bass_kernel_spmd
    nc = _get_nc()
    maps = _in_maps(x, W_attn, b_attn, W_proj, b_proj)
    res = run_bass_kernel_spmd(nc, maps, list(range(8)), trace=trace)
    out = np.empty((B, T, C), np.float32)
    for i in range(8):
        b_, r = i // 4, i % 4
        out[b_, r * TQ:(r + 1) * TQ, :] = res.results[i]["out"].astype(np.float32)
    # v-bias and proj-bias fold: softmax rows sum to 1, so
    # P @ (V + 1 b_v^T) = P @ V + b_v  ->  out += b_v @ W_proj + b_proj  (exact)
    b_attn = np.asarray(b_attn, np.float32)
    b_proj = np.asarray(b_proj, np.float32)
    if b_attn[2 * C:].any() or b_proj.any():
        out += (b_attn[2 * C:] @ np.asarray(W_proj, np.float32) + b_proj).astype(np.float32)
    return out, res


def kernel(x, W_attn, b_attn, W_proj, b_proj):
    out, _ = run(x, W_attn, b_attn, W_proj, b_proj, trace=False)
    return out


# revision 32
# speedup vs baseline: 1.0082x; 1.0082x over previous
"""Trainium2 Bass kernel for nn_AudioSelfAttention (B=2, T=2048, C=1024, H=16).

Sharding: sequence-parallel over the 8 NeuronCores. Core i handles batch
i//4 and query-token slice (i%4)*512. Each core computes K/V for its full
batch locally (redundant within the 4-core batch group — measured collective
cost here, ~76us floor + ~47us/MB, makes the AllGather alternative slower
than recomputation), computes attention for its own 512 query rows over all
16 heads, and the output projection. No collectives; the full output is
assembled on the host from the 8 row-shards.

The per-core xt input is ROLLED on the host so the core's own query block is
token-chunk 0 (attention is permutation-invariant over keys): q is computed
straight from xt chunk 0 and no separate xq input is needed.

Engine balance: ScalarE's exp stream (~1.25us per kt-chunk) is the pair-phase
floor, so pair 0's attention chunks are processed INLINE with the K0/K1/V
streaming phase (PE-heavy, exp fully hidden), and pairs 1-7 each drip-feed
the NEXT pair's K projection (2 matmuls per chunk) plus a spread schedule of
proj-partial groups, sized so every pair stays just under the exp pace.
xt is DMA'd in token chunks (chunk 0 feature-chunk-wise) so the Q phase
starts after ~130KB. The sync DMA queue carries only small weight blocks — a
descriptor-ring-full stall on the sync engine blocks tile-pool boundaries
and thus the PE — and all inputs are pre-shuffled on the host to
partition-major layouts so every DMA trigger generates only ~128 descriptors.

Compute dtype: bf16 matmul operands (fp32 matmul is 4 cycles/row on TRN2's
PE vs 1 for bf16), fp32 PSUM accumulation and softmax statistics. Output is
stored bf16 (quantization ~0.4% rel, well within budget) and upcast on host.

Layouts: activations live in transposed [feature, token] space so every
matmul contracts along partitions. Attention scores are computed as
S^T = K^T-chunks x Q^T (kt on partitions), softmax-exp runs on ScalarE over
4-bank PSUM spans, and the softmax denominator falls out of the y-matmul by
augmenting V with a ones-column (M=65). Per-pair normalization uses two tiny
half-broadcast matmuls (ones-row stationary) to spread 1/sum across
partitions. The v-part and proj biases are folded in exactly on the host
(softmax rows sum to 1, so they reduce to a constant row added to the
output); q/k biases are applied on-device in the PSUM->SBUF copies.
All proj contributions except the last feature chunk are accumulated during
the final two head-pairs, so the post-attention tail is just the last-pair
normalization, 8 single matmuls, and the (bf16) output DMA.
"""
import numpy as np

_CACHE = {}

B, T, C, H, D = 2, 2048, 1024, 16, 64
TQ = T * B // 8          # 512 query tokens per core
CC = C // 128            # 8 contraction chunks
NPAIR = H // 2           # 8 head pairs
NKT = T // 128           # 16 kt chunks


def _build_nc():
    import concourse.bacc as bacc
    import concourse.tile as tile
    import concourse.mybir as mybir

    f32 = mybir.dt.float32
    bf16 = mybir.dt.bfloat16
    Exp = mybir.ActivationFunctionType.Exp

    nc = bacc.Bacc(None, num_devices=8)
    # All inputs are pre-shuffled on the host into partition-major layouts so
    # every DMA is ~128 contiguous per-partition segments (cheap descriptor
    # generation — a (c p)-rearranged DMA costs 1024 descriptors and 8-28us
    # of trigger time on the issuing engine).
    # xt[p, cc, t]: rolled so this core's query block is token chunk 0
    xt = nc.declare_dram_parameter("xt", [128, CC, T], bf16, isOutput=False)
    # wqk[jc, p, cc, j] = W_attn[cc*128+p, jc*128+j]  (jc 0..7 = q, 8..15 = k)
    wqk = nc.declare_dram_parameter("wqk", [16, 128, CC, 128], bf16, isOutput=False)
    wv_in = nc.declare_dram_parameter("wv_in", [128, 2, CC, TQ], bf16, isOutput=False)
    ba = nc.declare_dram_parameter("ba", [128, 16], f32, isOutput=False)
    wp = nc.declare_dram_parameter("wp", [128, CC, C], bf16, isOutput=False)
    out = nc.declare_dram_parameter("out", [TQ, C], bf16, isOutput=True)

    with tile.TileContext(nc) as tc:
        with (
            tc.tile_pool(name="big", bufs=1) as big,
            tc.tile_pool(name="wst", bufs=3) as wst,
            tc.tile_pool(name="kpool", bufs=4) as kpool,
            tc.tile_pool(name="pexp", bufs=6) as pexp,
            tc.tile_pool(name="small", bufs=2) as small,
            tc.tile_pool(name="mmps", bufs=2, space="PSUM") as mmps,
            tc.tile_pool(name="spool", bufs=2, space="PSUM") as spool,
            tc.tile_pool(name="ypool", bufs=1, space="PSUM") as ypool,
        ):
            # ---- DMA schedule.
            # scalar queue: xt chunk 0 (cc-wise, so the Q phase starts after
            #   ~130KB), wv in dh halves, wp.
            # sync queue: wq blocks + ba, wk01, xt token chunks 1-3 (cc-wise
            #   contiguous tails), then per-pair wk drip later. ----
            xt_sb = big.tile([128, CC, T], bf16)
            for cc in range(CC):
                nc.scalar.dma_start(xt_sb[:, cc, 0:TQ], xt[:, cc, 0:TQ])

            # wq shares its slot with the proj partial accumulator (disjoint lifetimes)
            wq_all = big.tile([128, CC, CC, 128], bf16, tag="scratch16")
            for jc in range(2):
                nc.sync.dma_start(wq_all[:, jc, :, :], wqk[jc])
            ba_sb = big.tile([128, 16], f32)
            nc.sync.dma_start(ba_sb[:], ba[:])
            wk0 = wst.tile([128, CC, 128], bf16, tag="w", name="wk_0")
            nc.sync.dma_start(wk0[:], wqk[8])
            for jc in range(2, CC):
                nc.sync.dma_start(wq_all[:, jc, :, :], wqk[jc])
            wk1 = wst.tile([128, CC, 128], bf16, tag="w", name="wk_1")
            nc.sync.dma_start(wk1[:], wqk[9])

            # bulk tails ride the scalar queue — the sync queue stays lean so
            # its engine never hits a ring-full stall (tile-pool boundaries
            # ride the sync engine and a stalled trigger there blocks the PE)
            wv_sb = big.tile([128, 2, CC, TQ], bf16)
            nc.scalar.dma_start(wv_sb[:, 0], wv_in[:, 0])
            for cc in range(CC):
                nc.scalar.dma_start(xt_sb[:, cc, TQ:T], xt[:, cc, TQ:T])
            # dh1 half of wv is consumed only by the V drip in pairs 1-3
            nc.scalar.dma_start(wv_sb[:, 1], wv_in[:, 1])
            wp_sb = big.tile([128, CC, C], bf16)
            nc.scalar.dma_start(wp_sb[:], wp[:])

            sel_sb = big.tile([1, 64], bf16)
            nc.vector.memset(sel_sb[:], 1.0)
            q_sb = big.tile([128, CC, TQ], bf16)
            # v pair blocks padded to 144 cols (288B, 16B-aligned for both
            # head slices): [vA(64) | 1 | pad(7) | vB(64) | 1 | pad(7)].
            # One extra pad block lets M=128 stationary reads over-run (junk
            # cols only land in unread PSUM partitions 65..127).
            v_sb = big.tile([128, NKT, NPAIR + 1, 144], bf16)
            nc.vector.memset(
                v_sb.rearrange("p t r (h f) -> p t r h f", h=2)[:, :, :, :, 64:65], 1.0
            )
            yt_sb = big.tile([128, CC, TQ], bf16)
            yhat_sb = big.tile([128, CC, TQ], bf16)

            # ---- PE warm-up: the PE needs ~3us of continuous work to reach
            # the 2.4GHz p-state; burn the DMA lead-in on dummy matmuls over
            # the first xt chunk (output discarded) ----
            warm_ps = mmps.tile([128, TQ], f32, tag="mm", name="warm_ps")
            for w in range(8):
                nc.tensor.matmul(warm_ps[:], xt_sb[:, 0, 0:128],
                                 xt_sb[:, 0, 0:TQ], start=True, stop=True)

            # ---- Q phase: q^T[j, tq] = W_q^T @ xt[:, chunk0], cc-outer over
            # jc pairs so the first matmul needs only wq[0,1] + xt chunk0/cc0.
            # PSUM alternates mmps/ypool so groups don't stall on rotation. ----
            for jcg in range(CC // 2):
                jA, jB = 2 * jcg, 2 * jcg + 1
                if jcg % 2 == 0:
                    qA = mmps.tile([128, TQ], f32, tag="mm", name=f"qps_{jA}")
                    qB = mmps.tile([128, TQ], f32, tag="mm", name=f"qps_{jB}")
                else:
                    qAB = ypool.tile([128, 2 * TQ], f32, tag="y", name=f"qps_{jA}{jB}")
                    qA, qB = qAB[:, 0:TQ], qAB[:, TQ:2 * TQ]
                for cc in range(CC):
                    nc.tensor.matmul(qA[:], wq_all[:, jA, cc, :], xt_sb[:, cc, 0:TQ],
                                     start=(cc == 0), stop=(cc == CC - 1))
                    nc.tensor.matmul(qB[:], wq_all[:, jB, cc, :], xt_sb[:, cc, 0:TQ],
                                     start=(cc == 0), stop=(cc == CC - 1))
                nc.vector.tensor_scalar_add(q_sb[:, jA, :], qA[:], ba_sb[:, jA:jA + 1])
                nc.vector.tensor_scalar_add(q_sb[:, jB, :], qB[:], ba_sb[:, jB:jB + 1])

            # deferred per-pair normalization: emitted 3 chunks into the NEXT
            # pair so the sums->reciprocal chain never stalls the in-order PE
            # at a pair boundary
            s2_tiles = {}

            def emit_norm(p):
                s2 = s2_tiles.pop(p)
                r2 = small.tile([1, 2 * TQ], f32, tag="r2", name=f"r2_{p}", bufs=1)
                nc.vector.reciprocal_approx_fast(r2[:], s2[:])
                rb2 = small.tile([1, 2 * TQ], bf16, tag="rb2", name=f"rb2_{p}", bufs=1)
                nc.vector.tensor_copy(rb2[:], r2[:])
                bc = mmps.tile([128, TQ], f32, tag="mm", name=f"bc_{p}")
                nc.tensor.matmul(bc[0:64, :], sel_sb[:], rb2[:, 0:TQ], start=True,
                                 stop=True)
                nc.tensor.matmul(bc[64:128, :], sel_sb[:], rb2[:, TQ:2 * TQ],
                                 start=True, stop=True)
                nc.vector.tensor_mul(yt_sb[0:64, p, :], yhat_sb[0:64, p, :], bc[0:64, :])
                nc.vector.tensor_mul(yt_sb[64:128, p, :], yhat_sb[64:128, p, :],
                                     bc[64:128, :])

            def emit_drain(p, yAB):
                yA, yB = yAB[:, 0:TQ], yAB[:, TQ:2 * TQ]
                if p == NPAIR - 1:
                    # last pair: sums extracted on the (idle) ScalarE so the
                    # yhat copies run concurrently on vector
                    s2 = small.tile([1, 2 * TQ], f32, tag="s2", name=f"s2_{p}")
                    nc.scalar.activation(s2[:], yAB[64:65, :],
                                         mybir.ActivationFunctionType.Copy)
                    nc.vector.tensor_copy(yhat_sb[0:64, p, :], yA[0:64, :])
                    nc.vector.tensor_copy(yhat_sb[64:128, p, :], yB[0:64, :])
                    r2 = small.tile([1, 2 * TQ], f32, tag="r2", name=f"r2_{p}", bufs=1)
                    nc.vector.reciprocal_approx_fast(r2[:], s2[:])
                    rb2 = small.tile([1, 2 * TQ], bf16, tag="rb2", name=f"rb2_{p}",
                                     bufs=1)
                    nc.vector.tensor_copy(rb2[:], r2[:])
                    bc = mmps.tile([128, TQ], f32, tag="mm", name=f"bc_{p}")
                    nc.tensor.matmul(bc[0:64, :], sel_sb[:], rb2[:, 0:TQ], start=True,
                                     stop=True)
                    nc.tensor.matmul(bc[64:128, :], sel_sb[:], rb2[:, TQ:2 * TQ],
                                     start=True, stop=True)
                    nc.vector.tensor_mul(yt_sb[0:64, p, :], yhat_sb[0:64, p, :],
                                         bc[0:64, :])
                    nc.vector.tensor_mul(yt_sb[64:128, p, :], yhat_sb[64:128, p, :],
                                         bc[64:128, :])
                else:
                    # stash unnormalized y + sums, freeing the accumulator
                    s2 = small.tile([1, 2 * TQ], f32, tag="s2", name=f"s2_{p}")
                    nc.vector.tensor_copy(s2[:], yAB[64:65, :])
                    s2_tiles[p] = s2
                    nc.vector.tensor_copy(yhat_sb[0:64, p, :], yA[0:64, :])
                    nc.vector.tensor_copy(yhat_sb[64:128, p, :], yB[0:64, :])

            def s_chunk(p, c, k_t, q_col, pe_tiles, split_exp=False):
                tt, off = c // 4, (c % 4) * 128
                sp = spool.tile([128, 2 * TQ], f32, tag="s", name=f"s_{p}_{c}")
                nc.tensor.matmul(sp[:, 0:TQ], k_t[0:64, tt, off:off + 128],
                                 q_sb[0:64, q_col, :], start=True, stop=True)
                nc.tensor.matmul(sp[:, TQ:2 * TQ], k_t[64:128, tt, off:off + 128],
                                 q_sb[64:128, q_col, :], start=True, stop=True)
                pe_t = pexp.tile([128, 2 * TQ], bf16, tag="pe", name=f"pe_{p}_{c}")
                if split_exp:
                    # y(c,A) can start while the B half is still on ScalarE
                    nc.scalar.activation(pe_t[:, 0:TQ], sp[:, 0:TQ], Exp, scale=0.125)
                    nc.scalar.activation(pe_t[:, TQ:2 * TQ], sp[:, TQ:2 * TQ],
                                         Exp, scale=0.125)
                else:
                    nc.scalar.activation(pe_t[:], sp[:], Exp, scale=0.125)
                pe_tiles[c] = pe_t

            def make_emit_y(p, yA, yB, pe_tiles):
                def emit_y(c):
                    pe_t = pe_tiles.pop(c)
                    vflat = v_sb[:, c].rearrange("p r f -> p (r f)")
                    nc.tensor.matmul(yA[:], vflat[:, p * 144:p * 144 + 128],
                                     pe_t[:, 0:TQ], start=(c == 0), stop=(c == NKT - 1))
                    nc.tensor.matmul(yB[:], vflat[:, p * 144 + 72:p * 144 + 200],
                                     pe_t[:, TQ:2 * TQ],
                                     start=(c == 0), stop=(c == NKT - 1))
                return emit_y


            def emit_v(ci, dh):
                v_ps = mmps.tile([128, TQ], f32, tag="mm", name=f"vps_{ci}_{dh}")
                for cc in range(CC):
                    nc.tensor.matmul(v_ps[:], xt_sb[:, cc, ci * 128:(ci + 1) * 128],
                                     wv_sb[:, dh, cc, :],
                                     start=(cc == 0), stop=(cc == CC - 1))
                nc.vector.tensor_copy(
                    v_sb[:, ci, 4 * dh:4 * dh + 4, :]
                    .rearrange("p r (h f) -> p r h f", h=2)[:, :, :, 0:64],
                    v_ps.rearrange("p (r h f) -> p r h f", r=4, h=2),
                )

            # ---- phase 1b: stream K0/K1 + the dh0 half of V per token chunk,
            # processing pair 0's attention chunks inline — ScalarE's exp
            # stream hides under the V matmuls. ----
            kt0 = kpool.tile([128, 4, TQ], bf16, tag="kp", name="k_0")
            kt1 = kpool.tile([128, 4, TQ], bf16, tag="kp", name="k_1")
            yAB0 = ypool.tile([128, 2 * TQ], f32, tag="y", name="yAB_0")
            pe0 = {}
            emit_y0 = make_emit_y(0, yAB0[:, 0:TQ], yAB0[:, TQ:2 * TQ], pe0)
            for tt in range(4):
                for j, (wk_t, k_t) in enumerate(((wk0, kt0), (wk1, kt1))):
                    k_ps = mmps.tile([128, TQ], f32, tag="mm", name=f"kps_{j}_{tt}")
                    for cc in range(CC):
                        nc.tensor.matmul(k_ps[:], wk_t[:, cc, :],
                                         xt_sb[:, cc, tt * TQ:(tt + 1) * TQ],
                                         start=(cc == 0), stop=(cc == CC - 1))
                    nc.vector.tensor_scalar_add(k_t[:, tt, :], k_ps[:],
                                                ba_sb[:, 8 + j:9 + j])
                for ci in range(4 * tt, 4 * tt + 4):
                    emit_v(ci, 0)
                    s_chunk(0, ci, kt0, 0, pe0)
                    if ci >= 2:
                        emit_y0(ci - 2)
            emit_y0(NKT - 2)
            emit_y0(NKT - 1)
            emit_drain(0, yAB0)
            ktiles = {1: kt1}

            # proj partial accumulator (slot shared with wq_all, whose
            # lifetime ends after the Q phase)
            opart_sb = big.tile([128, CC, TQ], f32, tag="scratch16")

            # V-dh1 drip: pairs 4-7's V half is computed during the exp-paced
            # pairs 1-3 (one transient psum block per kn-ring window; pair 1
            # fits two). {pair: {c_slot: v_chunk}}
            _VDH1_SCHED = {
                1: {1: 0, 3: 1, 5: 2, 7: 3, 9: 4, 11: 5, 13: 6, 15: 7},
                2: {3: 8, 7: 9, 11: 10, 15: 11},
                3: {3: 12, 7: 13, 11: 14, 15: 15},
            }

            # proj-partial drip schedule: pair -> {c_slot: [(tile, cc_lo, single)]}
            # 3 groups per pair at c={5,9,13} for pairs 2-6 (inside the kn-drip
            # ring pattern), remainder in pair 7 which has no K drip.
            _PROJ_SCHED = {
                2: {5: [(0, 0, False)], 9: [(1, 0, False)], 13: [(2, 0, False)]},
                3: {5: [(3, 0, False)], 9: [(4, 0, False)], 13: [(5, 0, False)]},
                4: {5: [(6, 0, False)], 9: [(7, 0, False)], 13: [(0, 2, False)]},
                5: {5: [(1, 2, False)], 9: [(2, 2, False)], 13: [(3, 2, False)]},
                6: {5: [(4, 2, False)], 9: [(5, 2, False)], 13: [(6, 2, False)]},
                7: {3: [(7, 2, False)], 4: [(0, 4, False)], 5: [(1, 4, False)],
                    6: [(2, 4, False)], 7: [(3, 4, False)], 8: [(4, 4, False)],
                    9: [(5, 4, False)], 10: [(6, 4, False)], 11: [(7, 4, False)],
                    12: [(0, 6, True), (1, 6, True)],
                    13: [(2, 6, True), (3, 6, True)],
                    14: [(4, 6, True), (5, 6, True)],
                    15: [(6, 6, True), (7, 6, True)]},
            }

            # ---- pairs 1-7 ----
            # Pipeline per pair: per kt-chunk c emit S(c) -> exp(c) -> y(c-1),
            # with the next pair's K-chunk matmuls drip-fed 2 per chunk so the
            # PE always has exp-independent work while ScalarE runs.
            for p in range(1, NPAIR):
                kt_next = p + 1 if p + 1 < NPAIR else None
                knext_state = {}

                def emit_knext(ci, p=p, kt_next=kt_next, st=None):
                    # two accumulation matmuls of k(p+1) per kt chunk index ci
                    if kt_next is None:
                        return
                    st = knext_state
                    tt, ai = ci // 4, (ci % 4) * 2
                    if ai == 0:
                        st["wk"] = wst.tile([128, CC, 128], bf16, tag="w",
                                            name=f"wkn_{kt_next}_{tt}") if tt == 0 else st["wk"]
                        if tt == 0:
                            nc.sync.dma_start(st["wk"][:], wqk[8 + kt_next])
                            st["kt"] = kpool.tile([128, 4, TQ], bf16, tag="kp",
                                                  name=f"k_{kt_next}")
                        st["ps"] = mmps.tile([128, TQ], f32, tag="mm",
                                             name=f"kn_{kt_next}_{tt}")
                    for cc in (ai, ai + 1):
                        nc.tensor.matmul(st["ps"][:], st["wk"][:, cc, :],
                                         xt_sb[:, cc, tt * TQ:(tt + 1) * TQ],
                                         start=(cc == 0), stop=(cc == CC - 1))
                    if ai == 6:
                        nc.vector.tensor_scalar_add(st["kt"][:, tt, :], st["ps"][:],
                                                    ba_sb[:, 8 + kt_next:9 + kt_next])
                        if tt == 3:
                            ktiles[kt_next] = st["kt"]

                k_t = ktiles.pop(p)
                yAB = ypool.tile([128, 2 * TQ], f32, tag="y", name=f"yAB_{p}")
                pe_tiles = {}
                emit_y = make_emit_y(p, yAB[:, 0:TQ], yAB[:, TQ:2 * TQ], pe_tiles)

                # proj partials spread over pairs 2-7, sized so every pair
                # stays under the exp pace (~170ns/chunk of free PE): 2-cc
                # groups accumulated into opart as the yt columns become
                # available (yt[cc] needs norm(cc), done at pair cc+1 c==2).
                # PROJ_SCHED[p] = [(c_slot, tile, cc_lo or cc6-single)].
                def emit_projpart(c, p=p):
                    sched = _PROJ_SCHED.get(p)
                    if not sched or c not in sched:
                        return
                    for gi, (i, cc_lo, single) in enumerate(sched[c]):
                        tt, oh = i // 2, i % 2
                        pp = mmps.tile([128, TQ], f32, tag="mm",
                                       name=f"pp_{p}_{c}_{gi}")
                        nc.tensor.matmul(pp[:],
                                         yt_sb[:, cc_lo, tt * 128:(tt + 1) * 128],
                                         wp_sb[:, cc_lo, oh * TQ:(oh + 1) * TQ],
                                         start=True, stop=single)
                        if not single:
                            nc.tensor.matmul(
                                pp[:], yt_sb[:, cc_lo + 1, tt * 128:(tt + 1) * 128],
                                wp_sb[:, cc_lo + 1, oh * TQ:(oh + 1) * TQ],
                                start=False, stop=True)
                        if cc_lo == 0:
                            nc.vector.tensor_copy(opart_sb[:, i, :], pp[:])
                        else:
                            nc.vector.tensor_add(opart_sb[:, i, :], opart_sb[:, i, :],
                                                 pp[:])

                for c in range(NKT):
                    s_chunk(p, c, k_t, p, pe_tiles,
                            split_exp=(p == NPAIR - 1 and c == NKT - 1))
                    if c >= 2:
                        emit_y(c - 2)
                    emit_knext(c)
                    vci = _VDH1_SCHED.get(p, {}).get(c)
                    if vci is not None:
                        emit_v(vci, 1)
                    emit_projpart(c)
                    if c == 2:
                        emit_norm(p - 1)
                emit_y(NKT - 2)
                emit_y(NKT - 1)
                emit_drain(p, yAB)

            # ---- output projection: last contraction chunk + stashed partials.
            # Both oh-halves of a token row go into one (now idle) spool tile,
            # so the tail is 4 wide adds and 4 full-row DMAs. ----
            for tt in range(4):
                o_ps = spool.tile([128, 2 * TQ], f32, tag="s", name=f"ops_{tt}")
                for oh in range(2):
                    nc.tensor.matmul(o_ps[:, oh * TQ:(oh + 1) * TQ],
                                     yt_sb[:, CC - 1, tt * 128:(tt + 1) * 128],
                                     wp_sb[:, CC - 1, oh * TQ:(oh + 1) * TQ],
                                     start=True, stop=True)
                o_sb = small.tile([128, 2 * TQ], bf16, tag=f"osb{tt}", name=f"osb_{tt}",
                                  bufs=1)
                nc.vector.tensor_add(
                    o_sb[:], o_ps[:],
                    opart_sb[:, 2 * tt:2 * tt + 2, :].rearrange("p i t -> p (i t)"))
                dma_eng = (nc.sync, nc.scalar, nc.gpsimd)[tt % 3]
                dma_eng.dma_start(out[tt * 128:(tt + 1) * 128, :], o_sb[:])
    nc.compile()
    return nc


def _get_nc():
    if "nc" not in _CACHE:
        _CACHE["nc"] = _build_nc()
    return _CACHE["nc"]


def _in_maps(x, W_attn, b_attn, W_proj, b_proj):
    import ml_dtypes
    bf = ml_dtypes.bfloat16
    x = np.asarray(x, np.float32).reshape(B, T, C)
    W_attn = np.asarray(W_attn, np.float32)
    b_attn = np.asarray(b_attn, np.float32)
    W_proj = np.asarray(W_proj, np.float32)

    xt_all = [x[b_].T.astype(bf) for b_ in range(B)]
    # jc-major contiguous q/k weight blocks: wqk[jc, p, cc, j]
    wqk = np.ascontiguousarray(
        W_attn[:, :2 * C].reshape(CC, 128, 16, 128).transpose(2, 1, 0, 3)
    ).astype(bf)
    # wv[p, dh, cc, t] = W_v[cc*128+p, dh*512+t] — partition-major halves
    wv = np.ascontiguousarray(
        W_attn[:, 2 * C:].reshape(CC, 128, 2, TQ).transpose(1, 2, 0, 3)).astype(bf)
    # wp[p, cc, d] = W_proj[cc*128+p, d]
    wp = np.ascontiguousarray(
        W_proj.reshape(CC, 128, C).transpose(1, 0, 2)).astype(bf)
    # ba pre-shuffled to [128, 16] so the DMA is 128 contiguous 64B rows
    ba = np.ascontiguousarray(b_attn[:2 * C].reshape(16, 128).T.astype(np.float32))

    maps = []
    for i in range(8):
        b_, r = i // 4, i % 4
        xt_b = xt_all[b_]
        if r:
            # roll so this core's query block is token chunk 0 (attention is
            # permutation-invariant over keys)
            xt_b = np.concatenate([xt_b[:, r * TQ:], xt_b[:, :r * TQ]], axis=1)
        # xt[p, cc, t] = xt_b[cc*128+p, t] — partition-major
        xt_b = np.ascontiguousarray(xt_b.reshape(CC, 128, T).transpose(1, 0, 2))
        maps.append({"xt": xt_b, "wqk": wqk, "wv_in": wv, "ba": ba, "wp": wp})
    return maps


def run(x, W_attn, b_attn, W_proj, b_proj, trace=False):
    from concourse.bass_utils import run_bass_kernel_spmd
    nc = _get_nc()
    maps = _in_maps(x, W_attn, b_attn, W_proj, b_proj)
    res = run_bass_kernel_spmd(nc, maps, list(range(8)), trace=trace)
    out = np.empty((B, T, C), np.float32)
    for i in range(8):
        b_, r = i // 4, i % 4
        out[b_, r * TQ:(r + 1) * TQ, :] = res.results[i]["out"].astype(np.float32)
    # v-bias and proj-bias fold: softmax rows sum to 1, so
    # P @ (V + 1 b_v^T) = P @ V + b_v  ->  out += b_v @ W_proj + b_proj  (exact)
    b_attn = np.asarray(b_attn, np.float32)
    b_proj = np.asarray(b_proj, np.float32)
    if b_attn[2 * C:].any() or b_proj.any():
        out += (b_attn[2 * C:] @ np.asarray(W_proj, np.float32) + b_proj).astype(np.float32)
    return out, res


def kernel(x, W_attn, b_attn, W_proj, b_proj):
    out, _ = run(x, W_attn, b_attn, W_proj, b_proj, trace=False)
    return out


# revision 33
# speedup vs baseline: 1.0191x; 1.0108x over previous
"""Trainium2 Bass kernel for nn_AudioSelfAttention (B=2, T=2048, C=1024, H=16).

Sharding: sequence-parallel over the 8 NeuronCores. Core i handles batch
i//4 and query-token slice (i%4)*512. Each core computes K/V for its full
batch locally (redundant within the 4-core batch group — measured collective
cost here, ~76us floor + ~47us/MB, makes the AllGather alternative slower
than recomputation), computes attention for its own 512 query rows over all
16 heads, and the output projection. No collectives; the full output is
assembled on the host from the 8 row-shards.

The per-core xt input is ROLLED on the host so the core's own query block is
token-chunk 0 (attention is permutation-invariant over keys): q is computed
straight from xt chunk 0 and no separate xq input is needed.

Engine balance: ScalarE's exp stream (~1.25us per kt-chunk) is the pair-phase
floor, so pair 0's attention chunks are processed INLINE with the K0/K1/V
streaming phase (PE-heavy, exp fully hidden), and pairs 1-7 each drip-feed
the NEXT pair's K projection (2 matmuls per chunk) plus a spread schedule of
proj-partial groups, sized so every pair stays just under the exp pace.
xt is DMA'd in token chunks (chunk 0 feature-chunk-wise) so the Q phase
starts after ~130KB. The sync DMA queue carries only small weight blocks — a
descriptor-ring-full stall on the sync engine blocks tile-pool boundaries
and thus the PE — and all inputs are pre-shuffled on the host to
partition-major layouts so every DMA trigger generates only ~128 descriptors.

Compute dtype: bf16 matmul operands (fp32 matmul is 4 cycles/row on TRN2's
PE vs 1 for bf16), fp32 PSUM accumulation and softmax statistics. Output is
stored bf16 (quantization ~0.4% rel, well within budget) and upcast on host.

Layouts: activations live in transposed [feature, token] space so every
matmul contracts along partitions. Attention scores are computed as
S^T = K^T-chunks x Q^T (kt on partitions), softmax-exp runs on ScalarE over
4-bank PSUM spans, and the softmax denominator falls out of the y-matmul by
augmenting V with a ones-column (M=65). Per-pair normalization uses two tiny
half-broadcast matmuls (ones-row stationary) to spread 1/sum across
partitions. The v-part and proj biases are folded in exactly on the host
(softmax rows sum to 1, so they reduce to a constant row added to the
output); q/k biases are applied on-device in the PSUM->SBUF copies.
All proj contributions except the last feature chunk are accumulated during
the final two head-pairs, so the post-attention tail is just the last-pair
normalization, 8 single matmuls, and the (bf16) output DMA.
"""
import numpy as np

_CACHE = {}

B, T, C, H, D = 2, 2048, 1024, 16, 64
TQ = T * B // 8          # 512 query tokens per core
CC = C // 128            # 8 contraction chunks
NPAIR = H // 2           # 8 head pairs
NKT = T // 128           # 16 kt chunks


def _build_nc():
    import concourse.bacc as bacc
    import concourse.tile as tile
    import concourse.mybir as mybir

    f32 = mybir.dt.float32
    bf16 = mybir.dt.bfloat16
    Exp = mybir.ActivationFunctionType.Exp

    nc = bacc.Bacc(None, num_devices=8)
    # All inputs are pre-shuffled on the host into partition-major layouts so
    # every DMA is ~128 contiguous per-partition segments (cheap descriptor
    # generation — a (c p)-rearranged DMA costs 1024 descriptors and 8-28us
    # of trigger time on the issuing engine).
    # xt[p, cc, t]: rolled so this core's query block is token chunk 0
    xt = nc.declare_dram_parameter("xt", [128, CC, T], bf16, isOutput=False)
    # wqk[jc, p, cc, j] = W_attn[cc*128+p, jc*128+j]  (jc 0..7 = q, 8..15 = k)
    wqk = nc.declare_dram_parameter("wqk", [16, 128, CC, 128], bf16, isOutput=False)
    wv_in = nc.declare_dram_parameter("wv_in", [128, 2, CC, TQ], bf16, isOutput=False)
    ba = nc.declare_dram_parameter("ba", [128, 16], f32, isOutput=False)
    wp = nc.declare_dram_parameter("wp", [128, CC, C], bf16, isOutput=False)
    out = nc.declare_dram_parameter("out", [TQ, C], bf16, isOutput=True)

    with tile.TileContext(nc) as tc:
        with (
            tc.tile_pool(name="big", bufs=1) as big,
            tc.tile_pool(name="wst", bufs=3) as wst,
            tc.tile_pool(name="kpool", bufs=4) as kpool,
            tc.tile_pool(name="pexp", bufs=6) as pexp,
            tc.tile_pool(name="small", bufs=2) as small,
            tc.tile_pool(name="mmps", bufs=2, space="PSUM") as mmps,
            tc.tile_pool(name="spool", bufs=2, space="PSUM") as spool,
            tc.tile_pool(name="ypool", bufs=1, space="PSUM") as ypool,
        ):
            # ---- DMA schedule.
            # scalar queue: xt chunk 0 (cc-wise, so the Q phase starts after
            #   ~130KB), wv in dh halves, wp.
            # sync queue: wq blocks + ba, wk01, xt token chunks 1-3 (cc-wise
            #   contiguous tails), then per-pair wk drip later. ----
            xt_sb = big.tile([128, CC, T], bf16)
            for cc in range(CC):
                nc.scalar.dma_start(xt_sb[:, cc, 0:TQ], xt[:, cc, 0:TQ])

            # wq shares its slot with the proj partial accumulator (disjoint lifetimes)
            wq_all = big.tile([128, CC, CC, 128], bf16, tag="scratch16")
            for jc in range(2):
                nc.sync.dma_start(wq_all[:, jc, :, :], wqk[jc])
            ba_sb = big.tile([128, 16], f32)
            nc.sync.dma_start(ba_sb[:], ba[:])
            wk0 = wst.tile([128, CC, 128], bf16, tag="w", name="wk_0")
            nc.sync.dma_start(wk0[:], wqk[8])
            for jc in range(2, CC):
                nc.sync.dma_start(wq_all[:, jc, :, :], wqk[jc])
            wk1 = wst.tile([128, CC, 128], bf16, tag="w", name="wk_1")
            nc.sync.dma_start(wk1[:], wqk[9])

            # bulk tails ride the scalar queue — the sync queue stays lean so
            # its engine never hits a ring-full stall (tile-pool boundaries
            # ride the sync engine and a stalled trigger there blocks the PE)
            wv_sb = big.tile([128, 2, CC, TQ], bf16)
            nc.scalar.dma_start(wv_sb[:, 0], wv_in[:, 0])
            for cc in range(CC):
                nc.scalar.dma_start(xt_sb[:, cc, TQ:T], xt[:, cc, TQ:T])
            # dh1 half of wv is consumed only by the V drip in pairs 1-3
            nc.scalar.dma_start(wv_sb[:, 1], wv_in[:, 1])
            wp_sb = big.tile([128, CC, C], bf16)
            nc.scalar.dma_start(wp_sb[:], wp[:])

            sel_sb = big.tile([1, 64], bf16)
            nc.vector.memset(sel_sb[:], 1.0)
            q_sb = big.tile([128, CC, TQ], bf16)
            # v pair blocks padded to 144 cols (288B, 16B-aligned for both
            # head slices): [vA(64) | 1 | pad(7) | vB(64) | 1 | pad(7)].
            # One extra pad block lets M=128 stationary reads over-run (junk
            # cols only land in unread PSUM partitions 65..127).
            v_sb = big.tile([128, NKT, NPAIR + 1, 144], bf16)
            nc.vector.memset(
                v_sb.rearrange("p t r (h f) -> p t r h f", h=2)[:, :, :, :, 64:65], 1.0
            )
            yt_sb = big.tile([128, CC, TQ], bf16)
            yhat_sb = big.tile([128, CC, TQ], bf16)

            # ---- PE warm-up: the PE needs ~3us of continuous work to reach
            # the 2.4GHz p-state; burn the DMA lead-in on dummy matmuls over
            # the first xt chunk (output discarded) ----
            warm_ps = mmps.tile([128, TQ], f32, tag="mm", name="warm_ps")
            for w in range(8):
                nc.tensor.matmul(warm_ps[:], xt_sb[:, 0, 0:128],
                                 xt_sb[:, 0, 0:TQ], start=True, stop=True)

            # ---- Q phase: q^T[j, tq] = W_q^T @ xt[:, chunk0], cc-outer over
            # jc pairs so the first matmul needs only wq[0,1] + xt chunk0/cc0.
            # PSUM alternates mmps/ypool so groups don't stall on rotation. ----
            for jcg in range(CC // 2):
                jA, jB = 2 * jcg, 2 * jcg + 1
                if jcg % 2 == 0:
                    qA = mmps.tile([128, TQ], f32, tag="mm", name=f"qps_{jA}")
                    qB = mmps.tile([128, TQ], f32, tag="mm", name=f"qps_{jB}")
                else:
                    qAB = ypool.tile([128, 2 * TQ], f32, tag="y", name=f"qps_{jA}{jB}")
                    qA, qB = qAB[:, 0:TQ], qAB[:, TQ:2 * TQ]
                for cc in range(CC):
                    nc.tensor.matmul(qA[:], wq_all[:, jA, cc, :], xt_sb[:, cc, 0:TQ],
                                     start=(cc == 0), stop=(cc == CC - 1))
                    nc.tensor.matmul(qB[:], wq_all[:, jB, cc, :], xt_sb[:, cc, 0:TQ],
                                     start=(cc == 0), stop=(cc == CC - 1))
                nc.vector.tensor_scalar_add(q_sb[:, jA, :], qA[:], ba_sb[:, jA:jA + 1])
                nc.vector.tensor_scalar_add(q_sb[:, jB, :], qB[:], ba_sb[:, jB:jB + 1])

            # deferred per-pair normalization: emitted 3 chunks into the NEXT
            # pair so the sums->reciprocal chain never stalls the in-order PE
            # at a pair boundary
            s2_tiles = {}

            def emit_norm(p):
                s2 = s2_tiles.pop(p)
                r2 = small.tile([1, 2 * TQ], f32, tag="r2", name=f"r2_{p}", bufs=1)
                nc.vector.reciprocal_approx_fast(r2[:], s2[:])
                rb2 = small.tile([1, 2 * TQ], bf16, tag="rb2", name=f"rb2_{p}", bufs=1)
                nc.vector.tensor_copy(rb2[:], r2[:])
                bc = mmps.tile([128, TQ], f32, tag="mm", name=f"bc_{p}")
                nc.tensor.matmul(bc[0:64, :], sel_sb[:], rb2[:, 0:TQ], start=True,
                                 stop=True)
                nc.tensor.matmul(bc[64:128, :], sel_sb[:], rb2[:, TQ:2 * TQ],
                                 start=True, stop=True)
                nc.vector.tensor_mul(yt_sb[0:64, p, :], yhat_sb[0:64, p, :], bc[0:64, :])
                nc.vector.tensor_mul(yt_sb[64:128, p, :], yhat_sb[64:128, p, :],
                                     bc[64:128, :])

            def emit_drain(p, yAB):
                yA, yB = yAB[:, 0:TQ], yAB[:, TQ:2 * TQ]
                if p == NPAIR - 1:
                    # last pair: sums extracted on the (idle) ScalarE so the
                    # yhat copies run concurrently on vector
                    s2 = small.tile([1, 2 * TQ], f32, tag="s2", name=f"s2_{p}")
                    nc.scalar.activation(s2[:], yAB[64:65, :],
                                         mybir.ActivationFunctionType.Copy)
                    nc.vector.tensor_copy(yhat_sb[0:64, p, :], yA[0:64, :])
                    nc.vector.tensor_copy(yhat_sb[64:128, p, :], yB[0:64, :])
                    r2 = small.tile([1, 2 * TQ], f32, tag="r2", name=f"r2_{p}", bufs=1)
                    nc.vector.reciprocal_approx_fast(r2[:], s2[:])
                    rb2 = small.tile([1, 2 * TQ], bf16, tag="rb2", name=f"rb2_{p}",
                                     bufs=1)
                    nc.vector.tensor_copy(rb2[:], r2[:])
                    bc = mmps.tile([128, TQ], f32, tag="mm", name=f"bc_{p}")
                    nc.tensor.matmul(bc[0:64, :], sel_sb[:], rb2[:, 0:TQ], start=True,
                                     stop=True)
                    nc.tensor.matmul(bc[64:128, :], sel_sb[:], rb2[:, TQ:2 * TQ],
                                     start=True, stop=True)
                    nc.vector.tensor_mul(yt_sb[0:64, p, :], yhat_sb[0:64, p, :],
                                         bc[0:64, :])
                    nc.vector.tensor_mul(yt_sb[64:128, p, :], yhat_sb[64:128, p, :],
                                         bc[64:128, :])
                else:
                    # stash unnormalized y + sums, freeing the accumulator
                    s2 = small.tile([1, 2 * TQ], f32, tag="s2", name=f"s2_{p}")
                    nc.vector.tensor_copy(s2[:], yAB[64:65, :])
                    s2_tiles[p] = s2
                    nc.vector.tensor_copy(yhat_sb[0:64, p, :], yA[0:64, :])
                    nc.vector.tensor_copy(yhat_sb[64:128, p, :], yB[0:64, :])

            def s_chunk(p, c, k_t, q_col, pe_tiles, split_exp=False):
                tt, off = c // 4, (c % 4) * 128
                sp = spool.tile([128, 2 * TQ], f32, tag="s", name=f"s_{p}_{c}")
                nc.tensor.matmul(sp[:, 0:TQ], k_t[0:64, tt, off:off + 128],
                                 q_sb[0:64, q_col, :], start=True, stop=True)
                nc.tensor.matmul(sp[:, TQ:2 * TQ], k_t[64:128, tt, off:off + 128],
                                 q_sb[64:128, q_col, :], start=True, stop=True)
                pe_t = pexp.tile([128, 2 * TQ], bf16, tag="pe", name=f"pe_{p}_{c}")
                if split_exp:
                    # y(c,A) can start while the B half is still on ScalarE
                    nc.scalar.activation(pe_t[:, 0:TQ], sp[:, 0:TQ], Exp, scale=0.125)
                    nc.scalar.activation(pe_t[:, TQ:2 * TQ], sp[:, TQ:2 * TQ],
                                         Exp, scale=0.125)
                else:
                    nc.scalar.activation(pe_t[:], sp[:], Exp, scale=0.125)
                pe_tiles[c] = pe_t

            def make_emit_y(p, yA, yB, pe_tiles):
                def emit_y(c):
                    pe_t = pe_tiles.pop(c)
                    vflat = v_sb[:, c].rearrange("p r f -> p (r f)")
                    nc.tensor.matmul(yA[:], vflat[:, p * 144:p * 144 + 128],
                                     pe_t[:, 0:TQ], start=(c == 0), stop=(c == NKT - 1))
                    nc.tensor.matmul(yB[:], vflat[:, p * 144 + 72:p * 144 + 200],
                                     pe_t[:, TQ:2 * TQ],
                                     start=(c == 0), stop=(c == NKT - 1))
                return emit_y


            def emit_v(ci, dh):
                v_ps = mmps.tile([128, TQ], f32, tag="mm", name=f"vps_{ci}_{dh}")
                for cc in range(CC):
                    nc.tensor.matmul(v_ps[:], xt_sb[:, cc, ci * 128:(ci + 1) * 128],
                                     wv_sb[:, dh, cc, :],
                                     start=(cc == 0), stop=(cc == CC - 1))
                nc.vector.tensor_copy(
                    v_sb[:, ci, 4 * dh:4 * dh + 4, :]
                    .rearrange("p r (h f) -> p r h f", h=2)[:, :, :, 0:64],
                    v_ps.rearrange("p (r h f) -> p r h f", r=4, h=2),
                )

            # ---- phase 1b: stream K0/K1 + the dh0 half of V per token chunk,
            # processing pair 0's attention chunks inline — ScalarE's exp
            # stream hides under the V matmuls. ----
            kt0 = kpool.tile([128, 4, TQ], bf16, tag="kp", name="k_0")
            kt1 = kpool.tile([128, 4, TQ], bf16, tag="kp", name="k_1")
            yAB0 = ypool.tile([128, 2 * TQ], f32, tag="y", name="yAB_0")
            pe0 = {}
            emit_y0 = make_emit_y(0, yAB0[:, 0:TQ], yAB0[:, TQ:2 * TQ], pe0)
            for tt in range(4):
                for j, (wk_t, k_t) in enumerate(((wk0, kt0), (wk1, kt1))):
                    k_ps = mmps.tile([128, TQ], f32, tag="mm", name=f"kps_{j}_{tt}")
                    for cc in range(CC):
                        nc.tensor.matmul(k_ps[:], wk_t[:, cc, :],
                                         xt_sb[:, cc, tt * TQ:(tt + 1) * TQ],
                                         start=(cc == 0), stop=(cc == CC - 1))
                    nc.vector.tensor_scalar_add(k_t[:, tt, :], k_ps[:],
                                                ba_sb[:, 8 + j:9 + j])
                for ci in range(4 * tt, 4 * tt + 4):
                    emit_v(ci, 0)
                    s_chunk(0, ci, kt0, 0, pe0)
                    if ci >= 2:
                        emit_y0(ci - 2)
            emit_y0(NKT - 2)
            emit_y0(NKT - 1)
            emit_drain(0, yAB0)
            ktiles = {1: kt1}

            # proj partial accumulator (slot shared with wq_all, whose
            # lifetime ends after the Q phase)
            opart_sb = big.tile([128, CC, TQ], f32, tag="scratch16")

            # V-dh1 drip: pairs 4-7's V half is computed during the exp-paced
            # pairs 1-3 (one transient psum block per kn-ring window; pair 1
            # fits two). {pair: {c_slot: v_chunk}}
            _VDH1_SCHED = {
                1: {1: 0, 3: 1, 5: 2, 7: 3, 9: 4, 11: 5, 13: 6, 15: 7},
                2: {3: 8, 7: 9, 11: 10, 15: 11},
                3: {3: 12, 7: 13, 11: 14, 15: 15},
            }

            # proj-partial drip schedule: pair -> {c_slot: [(tile, cc_lo, single)]}
            # 3 groups per pair at c={5,9,13} for pairs 2-6 (inside the kn-drip
            # ring pattern), remainder in pair 7 which has no K drip.
            _PROJ_SCHED = {
                2: {5: [(0, 0, False)], 9: [(1, 0, False)], 13: [(2, 0, False)]},
                3: {5: [(3, 0, False)], 9: [(4, 0, False)], 13: [(5, 0, False)]},
                4: {5: [(6, 0, False)], 9: [(7, 0, False)], 13: [(0, 2, False)]},
                5: {5: [(1, 2, False)], 9: [(2, 2, False)], 13: [(3, 2, False)]},
                6: {5: [(4, 2, False)], 9: [(5, 2, False)], 13: [(6, 2, False)]},
                7: {3: [(7, 2, False)], 4: [(0, 4, False)], 5: [(1, 4, False)],
                    6: [(2, 4, False)], 7: [(3, 4, False)], 8: [(4, 4, False)],
                    9: [(5, 4, False)], 10: [(6, 4, False)], 11: [(7, 4, False)],
                    12: [(0, 6, True), (1, 6, True)],
                    13: [(2, 6, True), (3, 6, True)],
                    14: [(4, 6, True), (5, 6, True)],
                    15: [(6, 6, True), (7, 6, True)]},
            }

            # ---- pairs 1-7 ----
            # Pipeline per pair: per kt-chunk c emit S(c) -> exp(c) -> y(c-1),
            # with the next pair's K-chunk matmuls drip-fed 2 per chunk so the
            # PE always has exp-independent work while ScalarE runs.
            for p in range(1, NPAIR):
                kt_next = p + 1 if p + 1 < NPAIR else None
                knext_state = {}

                def emit_knext(ci, p=p, kt_next=kt_next, st=None):
                    # two accumulation matmuls of k(p+1) per kt chunk index ci
                    if kt_next is None:
                        return
                    st = knext_state
                    tt, ai = ci // 4, (ci % 4) * 2
                    if ai == 0:
                        st["wk"] = wst.tile([128, CC, 128], bf16, tag="w",
                                            name=f"wkn_{kt_next}_{tt}") if tt == 0 else st["wk"]
                        if tt == 0:
                            nc.sync.dma_start(st["wk"][:], wqk[8 + kt_next])
                            st["kt"] = kpool.tile([128, 4, TQ], bf16, tag="kp",
                                                  name=f"k_{kt_next}")
                        st["ps"] = mmps.tile([128, TQ], f32, tag="mm",
                                             name=f"kn_{kt_next}_{tt}")
                    for cc in (ai, ai + 1):
                        nc.tensor.matmul(st["ps"][:], st["wk"][:, cc, :],
                                         xt_sb[:, cc, tt * TQ:(tt + 1) * TQ],
                                         start=(cc == 0), stop=(cc == CC - 1))
                    if ai == 6:
                        nc.vector.tensor_scalar_add(st["kt"][:, tt, :], st["ps"][:],
                                                    ba_sb[:, 8 + kt_next:9 + kt_next])
                        if tt == 3:
                            ktiles[kt_next] = st["kt"]

                k_t = ktiles.pop(p)
                yAB = ypool.tile([128, 2 * TQ], f32, tag="y", name=f"yAB_{p}")
                pe_tiles = {}
                emit_y = make_emit_y(p, yAB[:, 0:TQ], yAB[:, TQ:2 * TQ], pe_tiles)

                # proj partials spread over pairs 2-7, sized so every pair
                # stays under the exp pace (~170ns/chunk of free PE): 2-cc
                # groups accumulated into opart as the yt columns become
                # available (yt[cc] needs norm(cc), done at pair cc+1 c==2).
                # PROJ_SCHED[p] = [(c_slot, tile, cc_lo or cc6-single)].
                def emit_projpart(c, p=p):
                    sched = _PROJ_SCHED.get(p)
                    if not sched or c not in sched:
                        return
                    for gi, (i, cc_lo, single) in enumerate(sched[c]):
                        tt, oh = i // 2, i % 2
                        pp = mmps.tile([128, TQ], f32, tag="mm",
                                       name=f"pp_{p}_{c}_{gi}")
                        nc.tensor.matmul(pp[:],
                                         yt_sb[:, cc_lo, tt * 128:(tt + 1) * 128],
                                         wp_sb[:, cc_lo, oh * TQ:(oh + 1) * TQ],
                                         start=True, stop=single)
                        if not single:
                            nc.tensor.matmul(
                                pp[:], yt_sb[:, cc_lo + 1, tt * 128:(tt + 1) * 128],
                                wp_sb[:, cc_lo + 1, oh * TQ:(oh + 1) * TQ],
                                start=False, stop=True)
                        if cc_lo == 0:
                            nc.vector.tensor_copy(opart_sb[:, i, :], pp[:])
                        else:
                            nc.vector.tensor_add(opart_sb[:, i, :], opart_sb[:, i, :],
                                                 pp[:])

                for c in range(NKT):
                    s_chunk(p, c, k_t, p, pe_tiles,
                            split_exp=(p == NPAIR - 1 and c == NKT - 1))
                    if c >= 2:
                        emit_y(c - 2)
                    emit_knext(c)
                    vci = _VDH1_SCHED.get(p, {}).get(c)
                    if vci is not None:
                        emit_v(vci, 1)
                    emit_projpart(c)
                    if c == 3:
                        # one chunk later than the boundary drain copies so the
                        # bc matmuls never reach the PE before the vector
                        # reciprocal chain has drained
                        emit_norm(p - 1)
                emit_y(NKT - 2)
                emit_y(NKT - 1)
                emit_drain(p, yAB)

            # ---- output projection: last contraction chunk + stashed partials.
            # Both oh-halves of a token row go into one (now idle) spool tile,
            # so the tail is 4 wide adds and 4 full-row DMAs. ----
            for tt in range(4):
                o_ps = spool.tile([128, 2 * TQ], f32, tag="s", name=f"ops_{tt}")
                for oh in range(2):
                    nc.tensor.matmul(o_ps[:, oh * TQ:(oh + 1) * TQ],
                                     yt_sb[:, CC - 1, tt * 128:(tt + 1) * 128],
                                     wp_sb[:, CC - 1, oh * TQ:(oh + 1) * TQ],
                                     start=True, stop=True)
                o_sb = small.tile([128, 2 * TQ], bf16, tag=f"osb{tt}", name=f"osb_{tt}",
                                  bufs=1)
                nc.vector.tensor_add(
                    o_sb[:], o_ps[:],
                    opart_sb[:, 2 * tt:2 * tt + 2, :].rearrange("p i t -> p (i t)"))
                dma_eng = (nc.sync, nc.scalar, nc.gpsimd)[tt % 3]
                dma_eng.dma_start(out[tt * 128:(tt + 1) * 128, :], o_sb[:])
    nc.compile()
    return nc


def _get_nc():
    if "nc" not in _CACHE:
        _CACHE["nc"] = _build_nc()
    return _CACHE["nc"]


def _in_maps(x, W_attn, b_attn, W_proj, b_proj):
    import ml_dtypes
    bf = ml_dtypes.bfloat16
    x = np.asarray(x, np.float32).reshape(B, T, C)
    W_attn = np.asarray(W_attn, np.float32)
    b_attn = np.asarray(b_attn, np.float32)
    W_proj = np.asarray(W_proj, np.float32)

    xt_all = [x[b_].T.astype(bf) for b_ in range(B)]
    # jc-major contiguous q/k weight blocks: wqk[jc, p, cc, j]
    wqk = np.ascontiguousarray(
        W_attn[:, :2 * C].reshape(CC, 128, 16, 128).transpose(2, 1, 0, 3)
    ).astype(bf)
    # wv[p, dh, cc, t] = W_v[cc*128+p, dh*512+t] — partition-major halves
    wv = np.ascontiguousarray(
        W_attn[:, 2 * C:].reshape(CC, 128, 2, TQ).transpose(1, 2, 0, 3)).astype(bf)
    # wp[p, cc, d] = W_proj[cc*128+p, d]
    wp = np.ascontiguousarray(
        W_proj.reshape(CC, 128, C).transpose(1, 0, 2)).astype(bf)
    # ba pre-shuffled to [128, 16] so the DMA is 128 contiguous 64B rows
    ba = np.ascontiguousarray(b_attn[:2 * C].reshape(16, 128).T.astype(np.float32))

    maps = []
    for i in range(8):
        b_, r = i // 4, i % 4
        xt_b = xt_all[b_]
        if r:
            # roll so this core's query block is token chunk 0 (attention is
            # permutation-invariant over keys)
            xt_b = np.concatenate([xt_b[:, r * TQ:], xt_b[:, :r * TQ]], axis=1)
        # xt[p, cc, t] = xt_b[cc*128+p, t] — partition-major
        xt_b = np.ascontiguousarray(xt_b.reshape(CC, 128, T).transpose(1, 0, 2))
        maps.append({"xt": xt_b, "wqk": wqk, "wv_in": wv, "ba": ba, "wp": wp})
    return maps


def run(x, W_attn, b_attn, W_proj, b_proj, trace=False):
    from concourse.bass_utils import run_bass_kernel_spmd
    nc = _get_nc()
    maps = _in_maps(x, W_attn, b_attn, W_proj, b_proj)
    res = run_bass_kernel_spmd(nc, maps, list(range(8)), trace=trace)
    out = np.empty((B, T, C), np.float32)
    for i in range(8):
        b_, r = i // 4, i % 4
        out[b_, r * TQ:(r + 1) * TQ, :] = res.results[i]["out"].astype(np.float32)
    # v-bias and proj-bias fold: softmax rows sum to 1, so
    # P @ (V + 1 b_v^T) = P @ V + b_v  ->  out += b_v @ W_proj + b_proj  (exact)
    b_attn = np.asarray(b_attn, np.float32)
    b_proj = np.asarray(b_proj, np.float32)
    if b_attn[2 * C:].any() or b_proj.any():
        out += (b_attn[2 * C:] @ np.asarray(W_proj, np.float32) + b_proj).astype(np.float32)
    return out, res


def kernel(x, W_attn, b_attn, W_proj, b_proj):
    out, _ = run(x, W_attn, b_attn, W_proj, b_proj, trace=False)
    return out


# revision 34
# speedup vs baseline: 1.0353x; 1.0159x over previous
"""Trainium2 Bass kernel for nn_AudioSelfAttention (B=2, T=2048, C=1024, H=16).

Sharding: sequence-parallel over the 8 NeuronCores. Core i handles batch
i//4 and query-token slice (i%4)*512. Each core computes K/V for its full
batch locally (redundant within the 4-core batch group — measured collective
cost here, ~76us floor + ~47us/MB, makes the AllGather alternative slower
than recomputation), computes attention for its own 512 query rows over all
16 heads, and the output projection. No collectives; the full output is
assembled on the host from the 8 row-shards.

The per-core xt input is ROLLED on the host so the core's own query block is
token-chunk 0 (attention is permutation-invariant over keys): q is computed
straight from xt chunk 0 and no separate xq input is needed.

Engine balance: ScalarE's exp stream (~1.25us per kt-chunk) is the pair-phase
floor, so pair 0's attention chunks are processed INLINE with the K0/K1/V
streaming phase (PE-heavy, exp fully hidden), and pairs 1-7 each drip-feed
the NEXT pair's K projection (2 matmuls per chunk) plus a spread schedule of
proj-partial groups, sized so every pair stays just under the exp pace.
xt is DMA'd in token chunks (chunk 0 feature-chunk-wise) so the Q phase
starts after ~130KB. The sync DMA queue carries only small weight blocks — a
descriptor-ring-full stall on the sync engine blocks tile-pool boundaries
and thus the PE — and all inputs are pre-shuffled on the host to
partition-major layouts so every DMA trigger generates only ~128 descriptors.

Compute dtype: bf16 matmul operands (fp32 matmul is 4 cycles/row on TRN2's
PE vs 1 for bf16), fp32 PSUM accumulation and softmax statistics. Output is
stored bf16 (quantization ~0.4% rel, well within budget) and upcast on host.

Layouts: activations live in transposed [feature, token] space so every
matmul contracts along partitions. Attention scores are computed as
S^T = K^T-chunks x Q^T (kt on partitions), softmax-exp runs on ScalarE over
4-bank PSUM spans, and the softmax denominator falls out of the y-matmul by
augmenting V with a ones-column (M=65). Per-pair normalization uses two tiny
half-broadcast matmuls (ones-row stationary) to spread 1/sum across
partitions. The v-part and proj biases are folded in exactly on the host
(softmax rows sum to 1, so they reduce to a constant row added to the
output); q/k biases are applied on-device in the PSUM->SBUF copies.
All proj contributions except the last feature chunk are accumulated during
the final two head-pairs, so the post-attention tail is just the last-pair
normalization, 8 single matmuls, and the (bf16) output DMA.
"""
import numpy as np

_CACHE = {}

B, T, C, H, D = 2, 2048, 1024, 16, 64
TQ = T * B // 8          # 512 query tokens per core
CC = C // 128            # 8 contraction chunks
NPAIR = H // 2           # 8 head pairs
NKT = T // 128           # 16 kt chunks


def _build_nc():
    import concourse.bacc as bacc
    import concourse.tile as tile
    import concourse.mybir as mybir

    f32 = mybir.dt.float32
    bf16 = mybir.dt.bfloat16
    Exp = mybir.ActivationFunctionType.Exp

    nc = bacc.Bacc(None, num_devices=8)
    # All inputs are pre-shuffled on the host into partition-major layouts so
    # every DMA is ~128 contiguous per-partition segments (cheap descriptor
    # generation — a (c p)-rearranged DMA costs 1024 descriptors and 8-28us
    # of trigger time on the issuing engine).
    # xt[p, cc, t]: rolled so this core's query block is token chunk 0
    xt = nc.declare_dram_parameter("xt", [128, CC, T], bf16, isOutput=False)
    # wqk[jc, p, cc, j] = W_attn[cc*128+p, jc*128+j]  (jc 0..7 = q, 8..15 = k)
    wqk = nc.declare_dram_parameter("wqk", [16, 128, CC, 128], bf16, isOutput=False)
    wv_in = nc.declare_dram_parameter("wv_in", [128, 2, CC, TQ], bf16, isOutput=False)
    ba = nc.declare_dram_parameter("ba", [128, 16], f32, isOutput=False)
    wp = nc.declare_dram_parameter("wp", [128, CC, C], bf16, isOutput=False)
    out = nc.declare_dram_parameter("out", [TQ, C], bf16, isOutput=True)

    with tile.TileContext(nc) as tc:
        with (
            tc.tile_pool(name="big", bufs=1) as big,
            tc.tile_pool(name="wst", bufs=3) as wst,
            tc.tile_pool(name="kpool", bufs=4) as kpool,
            tc.tile_pool(name="pexp", bufs=6) as pexp,
            tc.tile_pool(name="small", bufs=2) as small,
            tc.tile_pool(name="mmps", bufs=2, space="PSUM") as mmps,
            tc.tile_pool(name="spool", bufs=2, space="PSUM") as spool,
            tc.tile_pool(name="ypool", bufs=1, space="PSUM") as ypool,
        ):
            # ---- DMA schedule.
            # scalar queue: xt chunk 0 (cc-wise, so the Q phase starts after
            #   ~130KB), wv in dh halves, wp.
            # sync queue: wq blocks + ba, wk01, xt token chunks 1-3 (cc-wise
            #   contiguous tails), then per-pair wk drip later. ----
            xt_sb = big.tile([128, CC, T], bf16)
            for cc in range(CC):
                nc.scalar.dma_start(xt_sb[:, cc, 0:TQ], xt[:, cc, 0:TQ])

            # wq shares its slot with the proj partial accumulator (disjoint lifetimes)
            wq_all = big.tile([128, CC, CC, 128], bf16, tag="scratch16")
            for jc in range(2):
                nc.sync.dma_start(wq_all[:, jc, :, :], wqk[jc])
            ba_sb = big.tile([128, 16], f32)
            nc.sync.dma_start(ba_sb[:], ba[:])
            wk0 = wst.tile([128, CC, 128], bf16, tag="w", name="wk_0")
            nc.sync.dma_start(wk0[:], wqk[8])
            for jc in range(2, CC):
                nc.sync.dma_start(wq_all[:, jc, :, :], wqk[jc])
            wk1 = wst.tile([128, CC, 128], bf16, tag="w", name="wk_1")
            nc.sync.dma_start(wk1[:], wqk[9])

            # bulk tails ride the scalar queue — the sync queue stays lean so
            # its engine never hits a ring-full stall (tile-pool boundaries
            # ride the sync engine and a stalled trigger there blocks the PE)
            wv_sb = big.tile([128, 2, CC, TQ], bf16)
            nc.scalar.dma_start(wv_sb[:, 0], wv_in[:, 0])
            for cc in range(CC):
                nc.scalar.dma_start(xt_sb[:, cc, TQ:T], xt[:, cc, TQ:T])
            # dh1 half of wv is consumed only by the V drip in pairs 1-3
            nc.scalar.dma_start(wv_sb[:, 1], wv_in[:, 1])
            wp_sb = big.tile([128, CC, C], bf16)
            nc.scalar.dma_start(wp_sb[:], wp[:])

            sel_sb = big.tile([1, 64], bf16)
            nc.vector.memset(sel_sb[:], 1.0)
            q_sb = big.tile([128, CC, TQ], bf16)
            # v pair blocks padded to 144 cols (288B, 16B-aligned for both
            # head slices): [vA(64) | 1 | pad(7) | vB(64) | 1 | pad(7)].
            # One extra pad block lets M=128 stationary reads over-run (junk
            # cols only land in unread PSUM partitions 65..127).
            v_sb = big.tile([128, NKT, NPAIR + 1, 144], bf16)
            nc.vector.memset(
                v_sb.rearrange("p t r (h f) -> p t r h f", h=2)[:, :, :, :, 64:65], 1.0
            )
            yt_sb = big.tile([128, CC, TQ], bf16)
            yhat_sb = big.tile([128, CC, TQ], bf16)

            # ---- PE warm-up: the PE needs ~3us of continuous work to reach
            # the 2.4GHz p-state; burn the DMA lead-in on dummy matmuls over
            # the first xt chunk (output discarded) ----
            warm_ps = mmps.tile([128, TQ], f32, tag="mm", name="warm_ps")
            for w in range(24):
                nc.tensor.matmul(warm_ps[:, 0:65], xt_sb[:, 0, 0:128],
                                 xt_sb[:, 0, 0:65], start=True, stop=True)

            # ---- Q phase: q^T[j, tq] = W_q^T @ xt[:, chunk0], cc-outer over
            # jc pairs so the first matmul needs only wq[0,1] + xt chunk0/cc0.
            # PSUM alternates mmps/ypool so groups don't stall on rotation. ----
            for jcg in range(CC // 2):
                jA, jB = 2 * jcg, 2 * jcg + 1
                if jcg % 2 == 0:
                    qA = mmps.tile([128, TQ], f32, tag="mm", name=f"qps_{jA}")
                    qB = mmps.tile([128, TQ], f32, tag="mm", name=f"qps_{jB}")
                else:
                    qAB = ypool.tile([128, 2 * TQ], f32, tag="y", name=f"qps_{jA}{jB}")
                    qA, qB = qAB[:, 0:TQ], qAB[:, TQ:2 * TQ]
                for cc in range(CC):
                    nc.tensor.matmul(qA[:], wq_all[:, jA, cc, :], xt_sb[:, cc, 0:TQ],
                                     start=(cc == 0), stop=(cc == CC - 1))
                    nc.tensor.matmul(qB[:], wq_all[:, jB, cc, :], xt_sb[:, cc, 0:TQ],
                                     start=(cc == 0), stop=(cc == CC - 1))
                nc.vector.tensor_scalar_add(q_sb[:, jA, :], qA[:], ba_sb[:, jA:jA + 1])
                nc.vector.tensor_scalar_add(q_sb[:, jB, :], qB[:], ba_sb[:, jB:jB + 1])

            # deferred per-pair normalization: emitted 3 chunks into the NEXT
            # pair so the sums->reciprocal chain never stalls the in-order PE
            # at a pair boundary
            s2_tiles = {}

            def emit_norm(p):
                s2 = s2_tiles.pop(p)
                r2 = small.tile([1, 2 * TQ], f32, tag="r2", name=f"r2_{p}", bufs=1)
                nc.vector.reciprocal_approx_fast(r2[:], s2[:])
                rb2 = small.tile([1, 2 * TQ], bf16, tag="rb2", name=f"rb2_{p}", bufs=1)
                nc.vector.tensor_copy(rb2[:], r2[:])
                bc = mmps.tile([128, TQ], f32, tag="mm", name=f"bc_{p}")
                nc.tensor.matmul(bc[0:64, :], sel_sb[:], rb2[:, 0:TQ], start=True,
                                 stop=True)
                nc.tensor.matmul(bc[64:128, :], sel_sb[:], rb2[:, TQ:2 * TQ],
                                 start=True, stop=True)
                nc.vector.tensor_mul(yt_sb[0:64, p, :], yhat_sb[0:64, p, :], bc[0:64, :])
                nc.vector.tensor_mul(yt_sb[64:128, p, :], yhat_sb[64:128, p, :],
                                     bc[64:128, :])

            def emit_drain(p, yAB):
                yA, yB = yAB[:, 0:TQ], yAB[:, TQ:2 * TQ]
                if p == NPAIR - 1:
                    # last pair: sums extracted on the (idle) ScalarE so the
                    # yhat copies run concurrently on vector
                    s2 = small.tile([1, 2 * TQ], f32, tag="s2", name=f"s2_{p}")
                    nc.scalar.activation(s2[:], yAB[64:65, :],
                                         mybir.ActivationFunctionType.Copy)
                    nc.vector.tensor_copy(yhat_sb[0:64, p, :], yA[0:64, :])
                    nc.vector.tensor_copy(yhat_sb[64:128, p, :], yB[0:64, :])
                    r2 = small.tile([1, 2 * TQ], f32, tag="r2", name=f"r2_{p}", bufs=1)
                    nc.vector.reciprocal_approx_fast(r2[:], s2[:])
                    rb2 = small.tile([1, 2 * TQ], bf16, tag="rb2", name=f"rb2_{p}",
                                     bufs=1)
                    nc.vector.tensor_copy(rb2[:], r2[:])
                    bc = mmps.tile([128, TQ], f32, tag="mm", name=f"bc_{p}")
                    nc.tensor.matmul(bc[0:64, :], sel_sb[:], rb2[:, 0:TQ], start=True,
                                     stop=True)
                    nc.tensor.matmul(bc[64:128, :], sel_sb[:], rb2[:, TQ:2 * TQ],
                                     start=True, stop=True)
                    nc.vector.tensor_mul(yt_sb[0:64, p, :], yhat_sb[0:64, p, :],
                                         bc[0:64, :])
                    nc.vector.tensor_mul(yt_sb[64:128, p, :], yhat_sb[64:128, p, :],
                                         bc[64:128, :])
                else:
                    # stash unnormalized y + sums, freeing the accumulator
                    s2 = small.tile([1, 2 * TQ], f32, tag="s2", name=f"s2_{p}")
                    nc.vector.tensor_copy(s2[:], yAB[64:65, :])
                    s2_tiles[p] = s2
                    nc.vector.tensor_copy(yhat_sb[0:64, p, :], yA[0:64, :])
                    nc.vector.tensor_copy(yhat_sb[64:128, p, :], yB[0:64, :])

            def s_chunk(p, c, k_t, q_col, pe_tiles, split_exp=False):
                tt, off = c // 4, (c % 4) * 128
                sp = spool.tile([128, 2 * TQ], f32, tag="s", name=f"s_{p}_{c}")
                nc.tensor.matmul(sp[:, 0:TQ], k_t[0:64, tt, off:off + 128],
                                 q_sb[0:64, q_col, :], start=True, stop=True)
                nc.tensor.matmul(sp[:, TQ:2 * TQ], k_t[64:128, tt, off:off + 128],
                                 q_sb[64:128, q_col, :], start=True, stop=True)
                pe_t = pexp.tile([128, 2 * TQ], bf16, tag="pe", name=f"pe_{p}_{c}")
                if split_exp:
                    # y(c,A) can start while the B half is still on ScalarE
                    nc.scalar.activation(pe_t[:, 0:TQ], sp[:, 0:TQ], Exp, scale=0.125)
                    nc.scalar.activation(pe_t[:, TQ:2 * TQ], sp[:, TQ:2 * TQ],
                                         Exp, scale=0.125)
                else:
                    nc.scalar.activation(pe_t[:], sp[:], Exp, scale=0.125)
                pe_tiles[c] = pe_t

            def make_emit_y(p, yA, yB, pe_tiles):
                def emit_y(c):
                    pe_t = pe_tiles.pop(c)
                    vflat = v_sb[:, c].rearrange("p r f -> p (r f)")
                    nc.tensor.matmul(yA[:], vflat[:, p * 144:p * 144 + 128],
                                     pe_t[:, 0:TQ], start=(c == 0), stop=(c == NKT - 1))
                    nc.tensor.matmul(yB[:], vflat[:, p * 144 + 72:p * 144 + 200],
                                     pe_t[:, TQ:2 * TQ],
                                     start=(c == 0), stop=(c == NKT - 1))
                return emit_y


            def emit_v(ci, dh):
                v_ps = mmps.tile([128, TQ], f32, tag="mm", name=f"vps_{ci}_{dh}")
                for cc in range(CC):
                    nc.tensor.matmul(v_ps[:], xt_sb[:, cc, ci * 128:(ci + 1) * 128],
                                     wv_sb[:, dh, cc, :],
                                     start=(cc == 0), stop=(cc == CC - 1))
                nc.vector.tensor_copy(
                    v_sb[:, ci, 4 * dh:4 * dh + 4, :]
                    .rearrange("p r (h f) -> p r h f", h=2)[:, :, :, 0:64],
                    v_ps.rearrange("p (r h f) -> p r h f", r=4, h=2),
                )

            # ---- phase 1b: stream K0/K1 + the dh0 half of V per token chunk,
            # processing pair 0's attention chunks inline — ScalarE's exp
            # stream hides under the V matmuls. ----
            kt0 = kpool.tile([128, 4, TQ], bf16, tag="kp", name="k_0")
            kt1 = kpool.tile([128, 4, TQ], bf16, tag="kp", name="k_1")
            yAB0 = ypool.tile([128, 2 * TQ], f32, tag="y", name="yAB_0")
            pe0 = {}
            emit_y0 = make_emit_y(0, yAB0[:, 0:TQ], yAB0[:, TQ:2 * TQ], pe0)
            for tt in range(4):
                for j, (wk_t, k_t) in enumerate(((wk0, kt0), (wk1, kt1))):
                    k_ps = mmps.tile([128, TQ], f32, tag="mm", name=f"kps_{j}_{tt}")
                    for cc in range(CC):
                        nc.tensor.matmul(k_ps[:], wk_t[:, cc, :],
                                         xt_sb[:, cc, tt * TQ:(tt + 1) * TQ],
                                         start=(cc == 0), stop=(cc == CC - 1))
                    nc.vector.tensor_scalar_add(k_t[:, tt, :], k_ps[:],
                                                ba_sb[:, 8 + j:9 + j])
                for ci in range(4 * tt, 4 * tt + 4):
                    emit_v(ci, 0)
                    s_chunk(0, ci, kt0, 0, pe0)
                    if ci >= 2:
                        emit_y0(ci - 2)
            emit_y0(NKT - 2)
            emit_y0(NKT - 1)
            emit_drain(0, yAB0)
            ktiles = {1: kt1}

            # proj partial accumulator (slot shared with wq_all, whose
            # lifetime ends after the Q phase)
            opart_sb = big.tile([128, CC, TQ], f32, tag="scratch16")

            # V-dh1 drip: pairs 4-7's V half is computed during the exp-paced
            # pairs 1-3 (one transient psum block per kn-ring window; pair 1
            # fits two). {pair: {c_slot: v_chunk}}
            _VDH1_SCHED = {
                1: {1: 0, 3: 1, 5: 2, 7: 3, 9: 4, 11: 5, 13: 6, 15: 7},
                2: {3: 8, 7: 9, 11: 10, 15: 11},
                3: {3: 12, 7: 13, 11: 14, 15: 15},
            }

            # proj-partial drip schedule: pair -> {c_slot: [(tile, cc_lo, single)]}
            # 3 groups per pair at c={5,9,13} for pairs 2-6 (inside the kn-drip
            # ring pattern), remainder in pair 7 which has no K drip.
            _PROJ_SCHED = {
                2: {5: [(0, 0, False)], 9: [(1, 0, False)], 13: [(2, 0, False)]},
                3: {5: [(3, 0, False)], 9: [(4, 0, False)], 13: [(5, 0, False)]},
                4: {5: [(6, 0, False)], 9: [(7, 0, False)], 13: [(0, 2, False)]},
                5: {5: [(1, 2, False)], 9: [(2, 2, False)], 13: [(3, 2, False)]},
                6: {5: [(4, 2, False)], 9: [(5, 2, False)], 13: [(6, 2, False)]},
                7: {3: [(7, 2, False)], 4: [(0, 4, False)], 5: [(1, 4, False)],
                    6: [(2, 4, False)], 7: [(3, 4, False)], 8: [(4, 4, False)],
                    9: [(5, 4, False)], 10: [(6, 4, False)], 11: [(7, 4, False)],
                    12: [(0, 6, True), (1, 6, True)],
                    13: [(2, 6, True), (3, 6, True)],
                    14: [(4, 6, True), (5, 6, True)],
                    15: [(6, 6, True), (7, 6, True)]},
            }

            # ---- pairs 1-7 ----
            # Pipeline per pair: per kt-chunk c emit S(c) -> exp(c) -> y(c-1),
            # with the next pair's K-chunk matmuls drip-fed 2 per chunk so the
            # PE always has exp-independent work while ScalarE runs.
            for p in range(1, NPAIR):
                kt_next = p + 1 if p + 1 < NPAIR else None
                knext_state = {}

                def emit_knext(ci, p=p, kt_next=kt_next, st=None):
                    # two accumulation matmuls of k(p+1) per kt chunk index ci
                    if kt_next is None:
                        return
                    st = knext_state
                    tt, ai = ci // 4, (ci % 4) * 2
                    if ai == 0:
                        st["wk"] = wst.tile([128, CC, 128], bf16, tag="w",
                                            name=f"wkn_{kt_next}_{tt}") if tt == 0 else st["wk"]
                        if tt == 0:
                            nc.sync.dma_start(st["wk"][:], wqk[8 + kt_next])
                            st["kt"] = kpool.tile([128, 4, TQ], bf16, tag="kp",
                                                  name=f"k_{kt_next}")
                        st["ps"] = mmps.tile([128, TQ], f32, tag="mm",
                                             name=f"kn_{kt_next}_{tt}")
                    for cc in (ai, ai + 1):
                        nc.tensor.matmul(st["ps"][:], st["wk"][:, cc, :],
                                         xt_sb[:, cc, tt * TQ:(tt + 1) * TQ],
                                         start=(cc == 0), stop=(cc == CC - 1))
                    if ai == 6:
                        nc.vector.tensor_scalar_add(st["kt"][:, tt, :], st["ps"][:],
                                                    ba_sb[:, 8 + kt_next:9 + kt_next])
                        if tt == 3:
                            ktiles[kt_next] = st["kt"]

                k_t = ktiles.pop(p)
                yAB = ypool.tile([128, 2 * TQ], f32, tag="y", name=f"yAB_{p}")
                pe_tiles = {}
                emit_y = make_emit_y(p, yAB[:, 0:TQ], yAB[:, TQ:2 * TQ], pe_tiles)

                # proj partials spread over pairs 2-7, sized so every pair
                # stays under the exp pace (~170ns/chunk of free PE): 2-cc
                # groups accumulated into opart as the yt columns become
                # available (yt[cc] needs norm(cc), done at pair cc+1 c==2).
                # PROJ_SCHED[p] = [(c_slot, tile, cc_lo or cc6-single)].
                def emit_projpart(c, p=p):
                    sched = _PROJ_SCHED.get(p)
                    if not sched or c not in sched:
                        return
                    for gi, (i, cc_lo, single) in enumerate(sched[c]):
                        tt, oh = i // 2, i % 2
                        pp = mmps.tile([128, TQ], f32, tag="mm",
                                       name=f"pp_{p}_{c}_{gi}")
                        nc.tensor.matmul(pp[:],
                                         yt_sb[:, cc_lo, tt * 128:(tt + 1) * 128],
                                         wp_sb[:, cc_lo, oh * TQ:(oh + 1) * TQ],
                                         start=True, stop=single)
                        if not single:
                            nc.tensor.matmul(
                                pp[:], yt_sb[:, cc_lo + 1, tt * 128:(tt + 1) * 128],
                                wp_sb[:, cc_lo + 1, oh * TQ:(oh + 1) * TQ],
                                start=False, stop=True)
                        if cc_lo == 0:
                            nc.vector.tensor_copy(opart_sb[:, i, :], pp[:])
                        else:
                            nc.vector.tensor_add(opart_sb[:, i, :], opart_sb[:, i, :],
                                                 pp[:])

                for c in range(NKT):
                    s_chunk(p, c, k_t, p, pe_tiles,
                            split_exp=(p == NPAIR - 1 and c == NKT - 1))
                    if c >= 2:
                        emit_y(c - 2)
                    emit_knext(c)
                    vci = _VDH1_SCHED.get(p, {}).get(c)
                    if vci is not None:
                        emit_v(vci, 1)
                    emit_projpart(c)
                    if c == (5 if p == NPAIR - 1 else 3):
                        # past the boundary drain copies so the bc matmuls
                        # never reach the PE before the vector reciprocal
                        # chain has drained; pair 7 (no K drip to absorb
                        # stalls, norm output unused until c>=12) goes later
                        emit_norm(p - 1)
                emit_y(NKT - 2)
                emit_y(NKT - 1)
                emit_drain(p, yAB)

            # ---- output projection: last contraction chunk + stashed partials.
            # Both oh-halves of a token row go into one (now idle) spool tile,
            # so the tail is 4 wide adds and 4 full-row DMAs. ----
            for tt in range(4):
                o_ps = spool.tile([128, 2 * TQ], f32, tag="s", name=f"ops_{tt}")
                for oh in range(2):
                    nc.tensor.matmul(o_ps[:, oh * TQ:(oh + 1) * TQ],
                                     yt_sb[:, CC - 1, tt * 128:(tt + 1) * 128],
                                     wp_sb[:, CC - 1, oh * TQ:(oh + 1) * TQ],
                                     start=True, stop=True)
                o_sb = small.tile([128, 2 * TQ], bf16, tag=f"osb{tt}", name=f"osb_{tt}",
                                  bufs=1)
                nc.vector.tensor_add(
                    o_sb[:], o_ps[:],
                    opart_sb[:, 2 * tt:2 * tt + 2, :].rearrange("p i t -> p (i t)"))
                dma_eng = (nc.sync, nc.scalar, nc.gpsimd)[tt % 3]
                dma_eng.dma_start(out[tt * 128:(tt + 1) * 128, :], o_sb[:])
    nc.compile()
    return nc


def _get_nc():
    if "nc" not in _CACHE:
        _CACHE["nc"] = _build_nc()
    return _CACHE["nc"]


def _in_maps(x, W_attn, b_attn, W_proj, b_proj):
    import ml_dtypes
    bf = ml_dtypes.bfloat16
    x = np.asarray(x, np.float32).reshape(B, T, C)
    W_attn = np.asarray(W_attn, np.float32)
    b_attn = np.asarray(b_attn, np.float32)
    W_proj = np.asarray(W_proj, np.float32)

    xt_all = [x[b_].T.astype(bf) for b_ in range(B)]
    # jc-major contiguous q/k weight blocks: wqk[jc, p, cc, j]
    wqk = np.ascontiguousarray(
        W_attn[:, :2 * C].reshape(CC, 128, 16, 128).transpose(2, 1, 0, 3)
    ).astype(bf)
    # wv[p, dh, cc, t] = W_v[cc*128+p, dh*512+t] — partition-major halves
    wv = np.ascontiguousarray(
        W_attn[:, 2 * C:].reshape(CC, 128, 2, TQ).transpose(1, 2, 0, 3)).astype(bf)
    # wp[p, cc, d] = W_proj[cc*128+p, d]
    wp = np.ascontiguousarray(
        W_proj.reshape(CC, 128, C).transpose(1, 0, 2)).astype(bf)
    # ba pre-shuffled to [128, 16] so the DMA is 128 contiguous 64B rows
    ba = np.ascontiguousarray(b_attn[:2 * C].reshape(16, 128).T.astype(np.float32))

    maps = []
    for i in range(8):
        b_, r = i // 4, i % 4
        xt_b = xt_all[b_]
        if r:
            # roll so this core's query block is token chunk 0 (attention is
            # permutation-invariant over keys)
            xt_b = np.concatenate([xt_b[:, r * TQ:], xt_b[:, :r * TQ]], axis=1)
        # xt[p, cc, t] = xt_b[cc*128+p, t] — partition-major
        xt_b = np.ascontiguousarray(xt_b.reshape(CC, 128, T).transpose(1, 0, 2))
        maps.append({"xt": xt_b, "wqk": wqk, "wv_in": wv, "ba": ba, "wp": wp})
    return maps


def run(x, W_attn, b_attn, W_proj, b_proj, trace=False):
    from concourse.bass_utils import run_bass_kernel_spmd
    nc = _get_nc()
    maps = _in_maps(x, W_attn, b_attn, W_proj, b_proj)
    res = run_bass_kernel_spmd(nc, maps, list(range(8)), trace=trace)
    out = np.empty((B, T, C), np.float32)
    for i in range(8):
        b_, r = i // 4, i % 4
        out[b_, r * TQ:(r + 1) * TQ, :] = res.results[i]["out"].astype(np.float32)
    # v-bias and proj-bias fold: softmax rows sum to 1, so
    # P @ (V + 1 b_v^T) = P @ V + b_v  ->  out += b_v @ W_proj + b_proj  (exact)
    b_attn = np.asarray(b_attn, np.float32)
    b_proj = np.asarray(b_proj, np.float32)
    if b_attn[2 * C:].any() or b_proj.any():
        out += (b_attn[2 * C:] @ np.asarray(W_proj, np.float32) + b_proj).astype(np.float32)
    return out, res


def kernel(x, W_attn, b_attn, W_proj, b_proj):
    out, _ = run(x, W_attn, b_attn, W_proj, b_proj, trace=False)
    return out


# revision 35
# speedup vs baseline: 1.0384x; 1.0029x over previous
"""Trainium2 Bass kernel for nn_AudioSelfAttention (B=2, T=2048, C=1024, H=16).

Sharding: sequence-parallel over the 8 NeuronCores. Core i handles batch
i//4 and query-token slice (i%4)*512. Each core computes K/V for its full
batch locally (redundant within the 4-core batch group — measured collective
cost here, ~76us floor + ~47us/MB, makes the AllGather alternative slower
than recomputation), computes attention for its own 512 query rows over all
16 heads, and the output projection. No collectives; the full output is
assembled on the host from the 8 row-shards.

The per-core xt input is ROLLED on the host so the core's own query block is
token-chunk 0 (attention is permutation-invariant over keys): q is computed
straight from xt chunk 0 and no separate xq input is needed.

Engine balance: ScalarE's exp stream (~1.25us per kt-chunk) is the pair-phase
floor, so pair 0's attention chunks are processed INLINE with the K0/K1/V
streaming phase (PE-heavy, exp fully hidden), and pairs 1-7 each drip-feed
the NEXT pair's K projection (2 matmuls per chunk) plus a spread schedule of
proj-partial groups, sized so every pair stays just under the exp pace.
xt is DMA'd in token chunks (chunk 0 feature-chunk-wise) so the Q phase
starts after ~130KB. The sync DMA queue carries only small weight blocks — a
descriptor-ring-full stall on the sync engine blocks tile-pool boundaries
and thus the PE — and all inputs are pre-shuffled on the host to
partition-major layouts so every DMA trigger generates only ~128 descriptors.

Compute dtype: bf16 matmul operands (fp32 matmul is 4 cycles/row on TRN2's
PE vs 1 for bf16), fp32 PSUM accumulation and softmax statistics. Output is
stored bf16 (quantization ~0.4% rel, well within budget) and upcast on host.

Layouts: activations live in transposed [feature, token] space so every
matmul contracts along partitions. Attention scores are computed as
S^T = K^T-chunks x Q^T (kt on partitions), softmax-exp runs on ScalarE over
4-bank PSUM spans, and the softmax denominator falls out of the y-matmul by
augmenting V with a ones-column (M=65). Per-pair normalization uses two tiny
half-broadcast matmuls (ones-row stationary) to spread 1/sum across
partitions. The v-part and proj biases are folded in exactly on the host
(softmax rows sum to 1, so they reduce to a constant row added to the
output); q/k biases are applied on-device in the PSUM->SBUF copies.
All proj contributions except the last feature chunk are accumulated during
the final two head-pairs, so the post-attention tail is just the last-pair
normalization, 8 single matmuls, and the (bf16) output DMA.
"""
import numpy as np

_CACHE = {}

B, T, C, H, D = 2, 2048, 1024, 16, 64
TQ = T * B // 8          # 512 query tokens per core
CC = C // 128            # 8 contraction chunks
NPAIR = H // 2           # 8 head pairs
NKT = T // 128           # 16 kt chunks


def _build_nc():
    import concourse.bacc as bacc
    import concourse.tile as tile
    import concourse.mybir as mybir

    f32 = mybir.dt.float32
    bf16 = mybir.dt.bfloat16
    Exp = mybir.ActivationFunctionType.Exp

    nc = bacc.Bacc(None, num_devices=8)
    # All inputs are pre-shuffled on the host into partition-major layouts so
    # every DMA is ~128 contiguous per-partition segments (cheap descriptor
    # generation — a (c p)-rearranged DMA costs 1024 descriptors and 8-28us
    # of trigger time on the issuing engine).
    # xt[p, cc, t]: rolled so this core's query block is token chunk 0
    xt = nc.declare_dram_parameter("xt", [128, CC, T], bf16, isOutput=False)
    # wqk[jc, p, cc, j] = W_attn[cc*128+p, jc*128+j]  (jc 0..7 = q, 8..15 = k)
    wqk = nc.declare_dram_parameter("wqk", [16, 128, CC, 128], bf16, isOutput=False)
    wv_in = nc.declare_dram_parameter("wv_in", [128, 2, CC, TQ], bf16, isOutput=False)
    ba = nc.declare_dram_parameter("ba", [128, 16], f32, isOutput=False)
    wp = nc.declare_dram_parameter("wp", [128, CC, C], bf16, isOutput=False)
    out = nc.declare_dram_parameter("out", [TQ, C], bf16, isOutput=True)

    with tile.TileContext(nc) as tc:
        with (
            tc.tile_pool(name="big", bufs=1) as big,
            tc.tile_pool(name="wst", bufs=3) as wst,
            tc.tile_pool(name="kpool", bufs=4) as kpool,
            tc.tile_pool(name="pexp", bufs=6) as pexp,
            tc.tile_pool(name="small", bufs=2) as small,
            tc.tile_pool(name="mmps", bufs=2, space="PSUM") as mmps,
            tc.tile_pool(name="spool", bufs=2, space="PSUM") as spool,
            tc.tile_pool(name="ypool", bufs=1, space="PSUM") as ypool,
        ):
            # ---- DMA schedule.
            # scalar queue: xt chunk 0 (cc-wise, so the Q phase starts after
            #   ~130KB), wv in dh halves, wp.
            # sync queue: wq blocks + ba, wk01, xt token chunks 1-3 (cc-wise
            #   contiguous tails), then per-pair wk drip later. ----
            xt_sb = big.tile([128, CC, T], bf16)
            for cc in range(CC):
                nc.scalar.dma_start(xt_sb[:, cc, 0:TQ], xt[:, cc, 0:TQ])

            # wq shares its slot with the proj partial accumulator (disjoint lifetimes)
            wq_all = big.tile([128, CC, CC, 128], bf16, tag="scratch16")
            for jc in range(2):
                nc.sync.dma_start(wq_all[:, jc, :, :], wqk[jc])
            ba_sb = big.tile([128, 16], f32)
            nc.sync.dma_start(ba_sb[:], ba[:])
            # all wq blocks before wk01: the Q phase consumes wq pairs every
            # ~3.5us while wk isn't needed until K01(tt0) after Q ends
            for jc in range(2, CC):
                nc.sync.dma_start(wq_all[:, jc, :, :], wqk[jc])
            wk0 = wst.tile([128, CC, 128], bf16, tag="w", name="wk_0")
            nc.sync.dma_start(wk0[:], wqk[8])
            wk1 = wst.tile([128, CC, 128], bf16, tag="w", name="wk_1")
            nc.sync.dma_start(wk1[:], wqk[9])

            # bulk tails ride the scalar queue — the sync queue stays lean so
            # its engine never hits a ring-full stall (tile-pool boundaries
            # ride the sync engine and a stalled trigger there blocks the PE)
            wv_sb = big.tile([128, 2, CC, TQ], bf16)
            nc.scalar.dma_start(wv_sb[:, 0], wv_in[:, 0])
            for cc in range(CC):
                nc.scalar.dma_start(xt_sb[:, cc, TQ:T], xt[:, cc, TQ:T])
            # dh1 half of wv is consumed only by the V drip in pairs 1-3
            nc.scalar.dma_start(wv_sb[:, 1], wv_in[:, 1])
            wp_sb = big.tile([128, CC, C], bf16)
            nc.scalar.dma_start(wp_sb[:], wp[:])

            sel_sb = big.tile([1, 64], bf16)
            nc.vector.memset(sel_sb[:], 1.0)
            q_sb = big.tile([128, CC, TQ], bf16)
            # v pair blocks padded to 144 cols (288B, 16B-aligned for both
            # head slices): [vA(64) | 1 | pad(7) | vB(64) | 1 | pad(7)].
            # One extra pad block lets M=128 stationary reads over-run (junk
            # cols only land in unread PSUM partitions 65..127).
            v_sb = big.tile([128, NKT, NPAIR + 1, 144], bf16)
            nc.vector.memset(
                v_sb.rearrange("p t r (h f) -> p t r h f", h=2)[:, :, :, :, 64:65], 1.0
            )
            yt_sb = big.tile([128, CC, TQ], bf16)
            yhat_sb = big.tile([128, CC, TQ], bf16)

            # ---- PE warm-up: the PE needs ~3us of continuous work to reach
            # the 2.4GHz p-state; burn the DMA lead-in on dummy matmuls over
            # the first xt chunk (output discarded) ----
            warm_ps = mmps.tile([128, TQ], f32, tag="mm", name="warm_ps")
            for w in range(24):
                nc.tensor.matmul(warm_ps[:, 0:65], xt_sb[:, 0, 0:128],
                                 xt_sb[:, 0, 0:65], start=True, stop=True)

            # ---- Q phase: q^T[j, tq] = W_q^T @ xt[:, chunk0], cc-outer over
            # jc pairs so the first matmul needs only wq[0,1] + xt chunk0/cc0.
            # PSUM alternates mmps/ypool so groups don't stall on rotation. ----
            for jcg in range(CC // 2):
                jA, jB = 2 * jcg, 2 * jcg + 1
                if jcg % 2 == 0:
                    qA = mmps.tile([128, TQ], f32, tag="mm", name=f"qps_{jA}")
                    qB = mmps.tile([128, TQ], f32, tag="mm", name=f"qps_{jB}")
                else:
                    qAB = ypool.tile([128, 2 * TQ], f32, tag="y", name=f"qps_{jA}{jB}")
                    qA, qB = qAB[:, 0:TQ], qAB[:, TQ:2 * TQ]
                for cc in range(CC):
                    nc.tensor.matmul(qA[:], wq_all[:, jA, cc, :], xt_sb[:, cc, 0:TQ],
                                     start=(cc == 0), stop=(cc == CC - 1))
                    nc.tensor.matmul(qB[:], wq_all[:, jB, cc, :], xt_sb[:, cc, 0:TQ],
                                     start=(cc == 0), stop=(cc == CC - 1))
                nc.vector.tensor_scalar_add(q_sb[:, jA, :], qA[:], ba_sb[:, jA:jA + 1])
                nc.vector.tensor_scalar_add(q_sb[:, jB, :], qB[:], ba_sb[:, jB:jB + 1])

            # deferred per-pair normalization: emitted 3 chunks into the NEXT
            # pair so the sums->reciprocal chain never stalls the in-order PE
            # at a pair boundary
            s2_tiles = {}

            def emit_norm(p):
                s2 = s2_tiles.pop(p)
                r2 = small.tile([1, 2 * TQ], f32, tag="r2", name=f"r2_{p}", bufs=1)
                nc.vector.reciprocal_approx_fast(r2[:], s2[:])
                rb2 = small.tile([1, 2 * TQ], bf16, tag="rb2", name=f"rb2_{p}", bufs=1)
                nc.vector.tensor_copy(rb2[:], r2[:])
                bc = mmps.tile([128, TQ], f32, tag="mm", name=f"bc_{p}")
                nc.tensor.matmul(bc[0:64, :], sel_sb[:], rb2[:, 0:TQ], start=True,
                                 stop=True)
                nc.tensor.matmul(bc[64:128, :], sel_sb[:], rb2[:, TQ:2 * TQ],
                                 start=True, stop=True)
                nc.vector.tensor_mul(yt_sb[0:64, p, :], yhat_sb[0:64, p, :], bc[0:64, :])
                nc.vector.tensor_mul(yt_sb[64:128, p, :], yhat_sb[64:128, p, :],
                                     bc[64:128, :])

            def emit_drain(p, yAB):
                yA, yB = yAB[:, 0:TQ], yAB[:, TQ:2 * TQ]
                if p == NPAIR - 1:
                    # last pair: sums extracted on the (idle) ScalarE so the
                    # yhat copies run concurrently on vector
                    s2 = small.tile([1, 2 * TQ], f32, tag="s2", name=f"s2_{p}")
                    nc.scalar.activation(s2[:], yAB[64:65, :],
                                         mybir.ActivationFunctionType.Copy)
                    nc.vector.tensor_copy(yhat_sb[0:64, p, :], yA[0:64, :])
                    nc.vector.tensor_copy(yhat_sb[64:128, p, :], yB[0:64, :])
                    r2 = small.tile([1, 2 * TQ], f32, tag="r2", name=f"r2_{p}", bufs=1)
                    nc.vector.reciprocal_approx_fast(r2[:], s2[:])
                    rb2 = small.tile([1, 2 * TQ], bf16, tag="rb2", name=f"rb2_{p}",
                                     bufs=1)
                    nc.vector.tensor_copy(rb2[:], r2[:])
                    bc = mmps.tile([128, TQ], f32, tag="mm", name=f"bc_{p}")
                    nc.tensor.matmul(bc[0:64, :], sel_sb[:], rb2[:, 0:TQ], start=True,
                                     stop=True)
                    nc.tensor.matmul(bc[64:128, :], sel_sb[:], rb2[:, TQ:2 * TQ],
                                     start=True, stop=True)
                    nc.vector.tensor_mul(yt_sb[0:64, p, :], yhat_sb[0:64, p, :],
                                         bc[0:64, :])
                    nc.vector.tensor_mul(yt_sb[64:128, p, :], yhat_sb[64:128, p, :],
                                         bc[64:128, :])
                else:
                    # stash unnormalized y + sums, freeing the accumulator
                    s2 = small.tile([1, 2 * TQ], f32, tag="s2", name=f"s2_{p}")
                    nc.vector.tensor_copy(s2[:], yAB[64:65, :])
                    s2_tiles[p] = s2
                    nc.vector.tensor_copy(yhat_sb[0:64, p, :], yA[0:64, :])
                    nc.vector.tensor_copy(yhat_sb[64:128, p, :], yB[0:64, :])

            def s_chunk(p, c, k_t, q_col, pe_tiles, split_exp=False):
                tt, off = c // 4, (c % 4) * 128
                sp = spool.tile([128, 2 * TQ], f32, tag="s", name=f"s_{p}_{c}")
                nc.tensor.matmul(sp[:, 0:TQ], k_t[0:64, tt, off:off + 128],
                                 q_sb[0:64, q_col, :], start=True, stop=True)
                nc.tensor.matmul(sp[:, TQ:2 * TQ], k_t[64:128, tt, off:off + 128],
                                 q_sb[64:128, q_col, :], start=True, stop=True)
                pe_t = pexp.tile([128, 2 * TQ], bf16, tag="pe", name=f"pe_{p}_{c}")
                if split_exp:
                    # y(c,A) can start while the B half is still on ScalarE
                    nc.scalar.activation(pe_t[:, 0:TQ], sp[:, 0:TQ], Exp, scale=0.125)
                    nc.scalar.activation(pe_t[:, TQ:2 * TQ], sp[:, TQ:2 * TQ],
                                         Exp, scale=0.125)
                else:
                    nc.scalar.activation(pe_t[:], sp[:], Exp, scale=0.125)
                pe_tiles[c] = pe_t

            def make_emit_y(p, yA, yB, pe_tiles):
                def emit_y(c):
                    pe_t = pe_tiles.pop(c)
                    vflat = v_sb[:, c].rearrange("p r f -> p (r f)")
                    nc.tensor.matmul(yA[:], vflat[:, p * 144:p * 144 + 128],
                                     pe_t[:, 0:TQ], start=(c == 0), stop=(c == NKT - 1))
                    nc.tensor.matmul(yB[:], vflat[:, p * 144 + 72:p * 144 + 200],
                                     pe_t[:, TQ:2 * TQ],
                                     start=(c == 0), stop=(c == NKT - 1))
                return emit_y


            def emit_v(ci, dh):
                v_ps = mmps.tile([128, TQ], f32, tag="mm", name=f"vps_{ci}_{dh}")
                for cc in range(CC):
                    nc.tensor.matmul(v_ps[:], xt_sb[:, cc, ci * 128:(ci + 1) * 128],
                                     wv_sb[:, dh, cc, :],
                                     start=(cc == 0), stop=(cc == CC - 1))
                nc.vector.tensor_copy(
                    v_sb[:, ci, 4 * dh:4 * dh + 4, :]
                    .rearrange("p r (h f) -> p r h f", h=2)[:, :, :, 0:64],
                    v_ps.rearrange("p (r h f) -> p r h f", r=4, h=2),
                )

            # ---- phase 1b: stream K0/K1 + the dh0 half of V per token chunk,
            # processing pair 0's attention chunks inline — ScalarE's exp
            # stream hides under the V matmuls. ----
            kt0 = kpool.tile([128, 4, TQ], bf16, tag="kp", name="k_0")
            kt1 = kpool.tile([128, 4, TQ], bf16, tag="kp", name="k_1")
            yAB0 = ypool.tile([128, 2 * TQ], f32, tag="y", name="yAB_0")
            pe0 = {}
            emit_y0 = make_emit_y(0, yAB0[:, 0:TQ], yAB0[:, TQ:2 * TQ], pe0)
            for tt in range(4):
                for j, (wk_t, k_t) in enumerate(((wk0, kt0), (wk1, kt1))):
                    k_ps = mmps.tile([128, TQ], f32, tag="mm", name=f"kps_{j}_{tt}")
                    for cc in range(CC):
                        nc.tensor.matmul(k_ps[:], wk_t[:, cc, :],
                                         xt_sb[:, cc, tt * TQ:(tt + 1) * TQ],
                                         start=(cc == 0), stop=(cc == CC - 1))
                    nc.vector.tensor_scalar_add(k_t[:, tt, :], k_ps[:],
                                                ba_sb[:, 8 + j:9 + j])
                for ci in range(4 * tt, 4 * tt + 4):
                    emit_v(ci, 0)
                    s_chunk(0, ci, kt0, 0, pe0)
                    if ci >= 2:
                        emit_y0(ci - 2)
            emit_y0(NKT - 2)
            emit_y0(NKT - 1)
            emit_drain(0, yAB0)
            ktiles = {1: kt1}

            # proj partial accumulator (slot shared with wq_all, whose
            # lifetime ends after the Q phase)
            opart_sb = big.tile([128, CC, TQ], f32, tag="scratch16")

            # V-dh1 drip: pairs 4-7's V half is computed during the exp-paced
            # pairs 1-3 (one transient psum block per kn-ring window; pair 1
            # fits two). {pair: {c_slot: v_chunk}}
            _VDH1_SCHED = {
                1: {1: 0, 3: 1, 5: 2, 7: 3, 9: 4, 11: 5, 13: 6, 15: 7},
                2: {3: 8, 7: 9, 11: 10, 15: 11},
                3: {3: 12, 7: 13, 11: 14, 15: 15},
            }

            # proj-partial drip schedule: pair -> {c_slot: [(tile, cc_lo, single)]}
            # 3 groups per pair at c={5,9,13} for pairs 2-6 (inside the kn-drip
            # ring pattern), remainder in pair 7 which has no K drip.
            _PROJ_SCHED = {
                2: {5: [(0, 0, False)], 9: [(1, 0, False)], 13: [(2, 0, False)]},
                3: {5: [(3, 0, False)], 9: [(4, 0, False)], 13: [(5, 0, False)]},
                4: {5: [(6, 0, False)], 9: [(7, 0, False)], 13: [(0, 2, False)]},
                5: {5: [(1, 2, False)], 9: [(2, 2, False)], 13: [(3, 2, False)]},
                6: {5: [(4, 2, False)], 9: [(5, 2, False)], 13: [(6, 2, False)]},
                7: {3: [(7, 2, False)], 4: [(0, 4, False)], 5: [(1, 4, False)],
                    6: [(2, 4, False)], 7: [(3, 4, False)], 8: [(4, 4, False)],
                    9: [(5, 4, False)], 10: [(6, 4, False)], 11: [(7, 4, False)],
                    12: [(0, 6, True), (1, 6, True)],
                    13: [(2, 6, True), (3, 6, True)],
                    14: [(4, 6, True), (5, 6, True)],
                    15: [(6, 6, True), (7, 6, True)]},
            }

            # ---- pairs 1-7 ----
            # Pipeline per pair: per kt-chunk c emit S(c) -> exp(c) -> y(c-1),
            # with the next pair's K-chunk matmuls drip-fed 2 per chunk so the
            # PE always has exp-independent work while ScalarE runs.
            for p in range(1, NPAIR):
                kt_next = p + 1 if p + 1 < NPAIR else None
                knext_state = {}

                def emit_knext(ci, p=p, kt_next=kt_next, st=None):
                    # two accumulation matmuls of k(p+1) per kt chunk index ci
                    if kt_next is None:
                        return
                    st = knext_state
                    tt, ai = ci // 4, (ci % 4) * 2
                    if ai == 0:
                        st["wk"] = wst.tile([128, CC, 128], bf16, tag="w",
                                            name=f"wkn_{kt_next}_{tt}") if tt == 0 else st["wk"]
                        if tt == 0:
                            nc.sync.dma_start(st["wk"][:], wqk[8 + kt_next])
                            st["kt"] = kpool.tile([128, 4, TQ], bf16, tag="kp",
                                                  name=f"k_{kt_next}")
                        st["ps"] = mmps.tile([128, TQ], f32, tag="mm",
                                             name=f"kn_{kt_next}_{tt}")
                    for cc in (ai, ai + 1):
                        nc.tensor.matmul(st["ps"][:], st["wk"][:, cc, :],
                                         xt_sb[:, cc, tt * TQ:(tt + 1) * TQ],
                                         start=(cc == 0), stop=(cc == CC - 1))
                    if ai == 6:
                        nc.vector.tensor_scalar_add(st["kt"][:, tt, :], st["ps"][:],
                                                    ba_sb[:, 8 + kt_next:9 + kt_next])
                        if tt == 3:
                            ktiles[kt_next] = st["kt"]

                k_t = ktiles.pop(p)
                yAB = ypool.tile([128, 2 * TQ], f32, tag="y", name=f"yAB_{p}")
                pe_tiles = {}
                emit_y = make_emit_y(p, yAB[:, 0:TQ], yAB[:, TQ:2 * TQ], pe_tiles)

                # proj partials spread over pairs 2-7, sized so every pair
                # stays under the exp pace (~170ns/chunk of free PE): 2-cc
                # groups accumulated into opart as the yt columns become
                # available (yt[cc] needs norm(cc), done at pair cc+1 c==2).
                # PROJ_SCHED[p] = [(c_slot, tile, cc_lo or cc6-single)].
                def emit_projpart(c, p=p):
                    sched = _PROJ_SCHED.get(p)
                    if not sched or c not in sched:
                        return
                    for gi, (i, cc_lo, single) in enumerate(sched[c]):
                        tt, oh = i // 2, i % 2
                        pp = mmps.tile([128, TQ], f32, tag="mm",
                                       name=f"pp_{p}_{c}_{gi}")
                        nc.tensor.matmul(pp[:],
                                         yt_sb[:, cc_lo, tt * 128:(tt + 1) * 128],
                                         wp_sb[:, cc_lo, oh * TQ:(oh + 1) * TQ],
                                         start=True, stop=single)
                        if not single:
                            nc.tensor.matmul(
                                pp[:], yt_sb[:, cc_lo + 1, tt * 128:(tt + 1) * 128],
                                wp_sb[:, cc_lo + 1, oh * TQ:(oh + 1) * TQ],
                                start=False, stop=True)
                        if cc_lo == 0:
                            nc.vector.tensor_copy(opart_sb[:, i, :], pp[:])
                        else:
                            nc.vector.tensor_add(opart_sb[:, i, :], opart_sb[:, i, :],
                                                 pp[:])

                for c in range(NKT):
                    s_chunk(p, c, k_t, p, pe_tiles,
                            split_exp=(p == NPAIR - 1 and c == NKT - 1))
                    if c >= 2:
                        emit_y(c - 2)
                    emit_knext(c)
                    vci = _VDH1_SCHED.get(p, {}).get(c)
                    if vci is not None:
                        emit_v(vci, 1)
                    emit_projpart(c)
                    if c == (5 if p == NPAIR - 1 else 3):
                        # past the boundary drain copies so the bc matmuls
                        # never reach the PE before the vector reciprocal
                        # chain has drained; pair 7 (no K drip to absorb
                        # stalls, norm output unused until c>=12) goes later
                        emit_norm(p - 1)
                emit_y(NKT - 2)
                emit_y(NKT - 1)
                emit_drain(p, yAB)

            # ---- output projection: last contraction chunk + stashed partials.
            # Both oh-halves of a token row go into one (now idle) spool tile,
            # so the tail is 4 wide adds and 4 full-row DMAs. ----
            for tt in range(4):
                o_ps = spool.tile([128, 2 * TQ], f32, tag="s", name=f"ops_{tt}")
                for oh in range(2):
                    nc.tensor.matmul(o_ps[:, oh * TQ:(oh + 1) * TQ],
                                     yt_sb[:, CC - 1, tt * 128:(tt + 1) * 128],
                                     wp_sb[:, CC - 1, oh * TQ:(oh + 1) * TQ],
                                     start=True, stop=True)
                o_sb = small.tile([128, 2 * TQ], bf16, tag=f"osb{tt}", name=f"osb_{tt}",
                                  bufs=1)
                nc.vector.tensor_add(
                    o_sb[:], o_ps[:],
                    opart_sb[:, 2 * tt:2 * tt + 2, :].rearrange("p i t -> p (i t)"))
                dma_eng = (nc.sync, nc.scalar, nc.gpsimd)[tt % 3]
                dma_eng.dma_start(out[tt * 128:(tt + 1) * 128, :], o_sb[:])
    nc.compile()
    return nc


def _get_nc():
    if "nc" not in _CACHE:
        _CACHE["nc"] = _build_nc()
    return _CACHE["nc"]


def _in_maps(x, W_attn, b_attn, W_proj, b_proj):
    import ml_dtypes
    bf = ml_dtypes.bfloat16
    x = np.asarray(x, np.float32).reshape(B, T, C)
    W_attn = np.asarray(W_attn, np.float32)
    b_attn = np.asarray(b_attn, np.float32)
    W_proj = np.asarray(W_proj, np.float32)

    xt_all = [x[b_].T.astype(bf) for b_ in range(B)]
    # jc-major contiguous q/k weight blocks: wqk[jc, p, cc, j]
    wqk = np.ascontiguousarray(
        W_attn[:, :2 * C].reshape(CC, 128, 16, 128).transpose(2, 1, 0, 3)
    ).astype(bf)
    # wv[p, dh, cc, t] = W_v[cc*128+p, dh*512+t] — partition-major halves
    wv = np.ascontiguousarray(
        W_attn[:, 2 * C:].reshape(CC, 128, 2, TQ).transpose(1, 2, 0, 3)).astype(bf)
    # wp[p, cc, d] = W_proj[cc*128+p, d]
    wp = np.ascontiguousarray(
        W_proj.reshape(CC, 128, C).transpose(1, 0, 2)).astype(bf)
    # ba pre-shuffled to [128, 16] so the DMA is 128 contiguous 64B rows
    ba = np.ascontiguousarray(b_attn[:2 * C].reshape(16, 128).T.astype(np.float32))

    maps = []
    for i in range(8):
        b_, r = i // 4, i % 4
        xt_b = xt_all[b_]
        if r:
            # roll so this core's query block is token chunk 0 (attention is
            # permutation-invariant over keys)
            xt_b = np.concatenate([xt_b[:, r * TQ:], xt_b[:, :r * TQ]], axis=1)
        # xt[p, cc, t] = xt_b[cc*128+p, t] — partition-major
        xt_b = np.ascontiguousarray(xt_b.reshape(CC, 128, T).transpose(1, 0, 2))
        maps.append({"xt": xt_b, "wqk": wqk, "wv_in": wv, "ba": ba, "wp": wp})
    return maps


def run(x, W_attn, b_attn, W_proj, b_proj, trace=False):
    from concourse.bass_utils import run_bass_kernel_spmd
    nc = _get_nc()
    maps = _in_maps(x, W_attn, b_attn, W_proj, b_proj)
    res = run_bass_kernel_spmd(nc, maps, list(range(8)), trace=trace)
    out = np.empty((B, T, C), np.float32)
    for i in range(8):
        b_, r = i // 4, i % 4
        out[b_, r * TQ:(r + 1) * TQ, :] = res.results[i]["out"].astype(np.float32)
    # v-bias and proj-bias fold: softmax rows sum to 1, so
    # P @ (V + 1 b_v^T) = P @ V + b_v  ->  out += b_v @ W_proj + b_proj  (exact)
    b_attn = np.asarray(b_attn, np.float32)
    b_proj = np.asarray(b_proj, np.float32)
    if b_attn[2 * C:].any() or b_proj.any():
        out += (b_attn[2 * C:] @ np.asarray(W_proj, np.float32) + b_proj).astype(np.float32)
    return out, res


def kernel(x, W_attn, b_attn, W_proj, b_proj):
    out, _ = run(x, W_attn, b_attn, W_proj, b_proj, trace=False)
    return out


# revision 36
# speedup vs baseline: 1.0435x; 1.0050x over previous
"""Trainium2 Bass kernel for nn_AudioSelfAttention (B=2, T=2048, C=1024, H=16).

Sharding: sequence-parallel over the 8 NeuronCores. Core i handles batch
i//4 and query-token slice (i%4)*512. Each core computes K/V for its full
batch locally (redundant within the 4-core batch group — measured collective
cost here, ~76us floor + ~47us/MB, makes the AllGather alternative slower
than recomputation), computes attention for its own 512 query rows over all
16 heads, and the output projection. No collectives; the full output is
assembled on the host from the 8 row-shards.

The per-core xt input is ROLLED on the host so the core's own query block is
token-chunk 0 (attention is permutation-invariant over keys): q is computed
straight from xt chunk 0 and no separate xq input is needed.

Engine balance: ScalarE's exp stream (~1.25us per kt-chunk) is the pair-phase
floor, so pair 0's attention chunks are processed INLINE with the K0/K1/V
streaming phase (PE-heavy, exp fully hidden), and pairs 1-7 each drip-feed
the NEXT pair's K projection (2 matmuls per chunk) plus a spread schedule of
proj-partial groups, sized so every pair stays just under the exp pace.
xt is DMA'd in token chunks (chunk 0 feature-chunk-wise) so the Q phase
starts after ~130KB. The sync DMA queue carries only small weight blocks — a
descriptor-ring-full stall on the sync engine blocks tile-pool boundaries
and thus the PE — and all inputs are pre-shuffled on the host to
partition-major layouts so every DMA trigger generates only ~128 descriptors.

Compute dtype: bf16 matmul operands (fp32 matmul is 4 cycles/row on TRN2's
PE vs 1 for bf16), fp32 PSUM accumulation and softmax statistics. Output is
stored bf16 (quantization ~0.4% rel, well within budget) and upcast on host.

Layouts: activations live in transposed [feature, token] space so every
matmul contracts along partitions. Attention scores are computed as
S^T = K^T-chunks x Q^T (kt on partitions), softmax-exp runs on ScalarE over
4-bank PSUM spans, and the softmax denominator falls out of the y-matmul by
augmenting V with a ones-column (M=65). Per-pair normalization uses two tiny
half-broadcast matmuls (ones-row stationary) to spread 1/sum across
partitions. The v-part and proj biases are folded in exactly on the host
(softmax rows sum to 1, so they reduce to a constant row added to the
output); q/k biases are applied on-device in the PSUM->SBUF copies.
All proj contributions except the last feature chunk are accumulated during
the final two head-pairs, so the post-attention tail is just the last-pair
normalization, 8 single matmuls, and the (bf16) output DMA.
"""
import numpy as np

_CACHE = {}

B, T, C, H, D = 2, 2048, 1024, 16, 64
TQ = T * B // 8          # 512 query tokens per core
CC = C // 128            # 8 contraction chunks
NPAIR = H // 2           # 8 head pairs
NKT = T // 128           # 16 kt chunks


def _build_nc():
    import concourse.bacc as bacc
    import concourse.tile as tile
    import concourse.mybir as mybir

    f32 = mybir.dt.float32
    bf16 = mybir.dt.bfloat16
    Exp = mybir.ActivationFunctionType.Exp

    nc = bacc.Bacc(None, num_devices=8)
    # All inputs are pre-shuffled on the host into partition-major layouts so
    # every DMA is ~128 contiguous per-partition segments (cheap descriptor
    # generation — a (c p)-rearranged DMA costs 1024 descriptors and 8-28us
    # of trigger time on the issuing engine).
    # xt[p, cc, t]: rolled so this core's query block is token chunk 0
    xt = nc.declare_dram_parameter("xt", [128, CC, T], bf16, isOutput=False)
    # wqk[jc, p, cc, j] = W_attn[cc*128+p, jc*128+j]  (jc 0..7 = q, 8..15 = k)
    wqk = nc.declare_dram_parameter("wqk", [16, 128, CC, 128], bf16, isOutput=False)
    wv_in = nc.declare_dram_parameter("wv_in", [128, 2, CC, TQ], bf16, isOutput=False)
    ba = nc.declare_dram_parameter("ba", [128, 16], f32, isOutput=False)
    wp = nc.declare_dram_parameter("wp", [128, CC, C], bf16, isOutput=False)
    out = nc.declare_dram_parameter("out", [TQ, C], bf16, isOutput=True)

    with tile.TileContext(nc) as tc:
        with (
            tc.tile_pool(name="big", bufs=1) as big,
            tc.tile_pool(name="wst", bufs=3) as wst,
            tc.tile_pool(name="kpool", bufs=4) as kpool,
            tc.tile_pool(name="pexp", bufs=6) as pexp,
            tc.tile_pool(name="small", bufs=2) as small,
            tc.tile_pool(name="mmps", bufs=2, space="PSUM") as mmps,
            tc.tile_pool(name="spool", bufs=2, space="PSUM") as spool,
            tc.tile_pool(name="ypool", bufs=1, space="PSUM") as ypool,
        ):
            # ---- DMA schedule.
            # scalar queue: xt chunk 0 (cc-wise, so the Q phase starts after
            #   ~130KB), wv in dh halves, wp.
            # sync queue: wq blocks + ba, wk01, xt token chunks 1-3 (cc-wise
            #   contiguous tails), then per-pair wk drip later. ----
            xt_sb = big.tile([128, CC, T], bf16)
            for cc in range(CC):
                nc.scalar.dma_start(xt_sb[:, cc, 0:TQ], xt[:, cc, 0:TQ])

            # wq shares its slot with the proj partial accumulator (disjoint lifetimes)
            wq_all = big.tile([128, CC, CC, 128], bf16, tag="scratch16")
            for jc in range(2):
                nc.sync.dma_start(wq_all[:, jc, :, :], wqk[jc])
            ba_sb = big.tile([128, 16], f32)
            nc.sync.dma_start(ba_sb[:], ba[:])
            # all wq blocks before wk01: the Q phase consumes wq pairs every
            # ~3.5us while wk isn't needed until K01(tt0) after Q ends
            for jc in range(2, CC):
                nc.sync.dma_start(wq_all[:, jc, :, :], wqk[jc])
            wk0 = wst.tile([128, CC, 128], bf16, tag="w", name="wk_0")
            nc.sync.dma_start(wk0[:], wqk[8])
            wk1 = wst.tile([128, CC, 128], bf16, tag="w", name="wk_1")
            nc.sync.dma_start(wk1[:], wqk[9])

            # bulk tails ride the scalar queue — the sync queue stays lean so
            # its engine never hits a ring-full stall (tile-pool boundaries
            # ride the sync engine and a stalled trigger there blocks the PE)
            wv_sb = big.tile([128, 2, CC, TQ], bf16)
            nc.scalar.dma_start(wv_sb[:, 0], wv_in[:, 0])
            for cc in range(CC):
                nc.scalar.dma_start(xt_sb[:, cc, TQ:T], xt[:, cc, TQ:T])
            # dh1 half of wv is consumed only by the V drip in pairs 1-3
            nc.scalar.dma_start(wv_sb[:, 1], wv_in[:, 1])
            wp_sb = big.tile([128, CC, C], bf16)
            nc.scalar.dma_start(wp_sb[:], wp[:])

            sel_sb = big.tile([1, 64], bf16)
            nc.vector.memset(sel_sb[:], 1.0)
            q_sb = big.tile([128, CC, TQ], bf16)
            # v pair blocks padded to 144 cols (288B, 16B-aligned for both
            # head slices): [vA(64) | 1 | pad(7) | vB(64) | 1 | pad(7)].
            # One extra pad block lets M=128 stationary reads over-run (junk
            # cols only land in unread PSUM partitions 65..127).
            v_sb = big.tile([128, NKT, NPAIR + 1, 144], bf16)
            nc.vector.memset(
                v_sb.rearrange("p t r (h f) -> p t r h f", h=2)[:, :, :, :, 64:65], 1.0
            )
            yt_sb = big.tile([128, CC, TQ], bf16)
            yhat_sb = big.tile([128, CC, TQ], bf16)

            # ---- PE warm-up: the PE needs ~3us of continuous work to reach
            # the 2.4GHz p-state; burn the DMA lead-in on dummy matmuls over
            # the first xt chunk (output discarded) ----
            warm_ps = mmps.tile([128, TQ], f32, tag="mm", name="warm_ps")
            for w in range(24):
                nc.tensor.matmul(warm_ps[:, 0:65], xt_sb[:, 0, 0:128],
                                 xt_sb[:, 0, 0:65], start=True, stop=True)

            # ---- Q phase: q^T[j, tq] = W_q^T @ xt[:, chunk0], cc-outer over
            # jc pairs so the first matmul needs only wq[0,1] + xt chunk0/cc0.
            # PSUM alternates mmps/ypool so groups don't stall on rotation. ----
            for jcg in range(CC // 2):
                jA, jB = 2 * jcg, 2 * jcg + 1
                if jcg % 2 == 0:
                    qA = mmps.tile([128, TQ], f32, tag="mm", name=f"qps_{jA}")
                    qB = mmps.tile([128, TQ], f32, tag="mm", name=f"qps_{jB}")
                else:
                    qAB = ypool.tile([128, 2 * TQ], f32, tag="y", name=f"qps_{jA}{jB}")
                    qA, qB = qAB[:, 0:TQ], qAB[:, TQ:2 * TQ]
                for cc in range(CC):
                    nc.tensor.matmul(qA[:], wq_all[:, jA, cc, :], xt_sb[:, cc, 0:TQ],
                                     start=(cc == 0), stop=(cc == CC - 1))
                    nc.tensor.matmul(qB[:], wq_all[:, jB, cc, :], xt_sb[:, cc, 0:TQ],
                                     start=(cc == 0), stop=(cc == CC - 1))
                nc.vector.tensor_scalar_add(q_sb[:, jA, :], qA[:], ba_sb[:, jA:jA + 1])
                nc.vector.tensor_scalar_add(q_sb[:, jB, :], qB[:], ba_sb[:, jB:jB + 1])

            # deferred per-pair normalization: emitted 3 chunks into the NEXT
            # pair so the sums->reciprocal chain never stalls the in-order PE
            # at a pair boundary
            s2_tiles = {}

            def emit_norm(p):
                s2 = s2_tiles.pop(p)
                r2 = small.tile([1, 2 * TQ], f32, tag="r2", name=f"r2_{p}", bufs=1)
                nc.vector.reciprocal_approx_fast(r2[:], s2[:])
                rb2 = small.tile([1, 2 * TQ], bf16, tag="rb2", name=f"rb2_{p}", bufs=1)
                nc.vector.tensor_copy(rb2[:], r2[:])
                bc = mmps.tile([128, TQ], f32, tag="mm", name=f"bc_{p}")
                nc.tensor.matmul(bc[0:64, :], sel_sb[:], rb2[:, 0:TQ], start=True,
                                 stop=True)
                nc.tensor.matmul(bc[64:128, :], sel_sb[:], rb2[:, TQ:2 * TQ],
                                 start=True, stop=True)
                nc.vector.tensor_mul(yt_sb[0:64, p, :], yhat_sb[0:64, p, :], bc[0:64, :])
                nc.vector.tensor_mul(yt_sb[64:128, p, :], yhat_sb[64:128, p, :],
                                     bc[64:128, :])

            def emit_drain(p, yAB):
                yA, yB = yAB[:, 0:TQ], yAB[:, TQ:2 * TQ]
                if p == NPAIR - 1:
                    # last pair: sums extracted on the (idle) ScalarE so the
                    # yhat copies run concurrently on vector
                    s2 = small.tile([1, 2 * TQ], f32, tag="s2", name=f"s2_{p}")
                    nc.scalar.activation(s2[:], yAB[64:65, :],
                                         mybir.ActivationFunctionType.Copy)
                    nc.vector.tensor_copy(yhat_sb[0:64, p, :], yA[0:64, :])
                    nc.vector.tensor_copy(yhat_sb[64:128, p, :], yB[0:64, :])
                    r2 = small.tile([1, 2 * TQ], f32, tag="r2", name=f"r2_{p}", bufs=1)
                    nc.vector.reciprocal_approx_fast(r2[:], s2[:])
                    rb2 = small.tile([1, 2 * TQ], bf16, tag="rb2", name=f"rb2_{p}",
                                     bufs=1)
                    nc.vector.tensor_copy(rb2[:], r2[:])
                    # keep the PE hot through the norm-chain wait (discarded
                    # matmuls) so the bc + final proj matmuls run at 2.4GHz
                    # instead of the 1.2GHz post-idle p-state
                    warm2 = mmps.tile([128, TQ], f32, tag="mm", name="warm_drain")
                    for w in range(14):
                        nc.tensor.matmul(warm2[:], xt_sb[:, 0, 0:128],
                                         xt_sb[:, 0, 0:TQ], start=True, stop=True)
                    bc = mmps.tile([128, TQ], f32, tag="mm", name=f"bc_{p}")
                    nc.tensor.matmul(bc[0:64, :], sel_sb[:], rb2[:, 0:TQ], start=True,
                                     stop=True)
                    nc.tensor.matmul(bc[64:128, :], sel_sb[:], rb2[:, TQ:2 * TQ],
                                     start=True, stop=True)
                    for w in range(6):
                        nc.tensor.matmul(warm2[:], xt_sb[:, 0, 0:128],
                                         xt_sb[:, 0, 0:TQ], start=True, stop=True)
                    nc.vector.tensor_mul(yt_sb[0:64, p, :], yhat_sb[0:64, p, :],
                                         bc[0:64, :])
                    nc.vector.tensor_mul(yt_sb[64:128, p, :], yhat_sb[64:128, p, :],
                                         bc[64:128, :])
                else:
                    # stash unnormalized y + sums, freeing the accumulator
                    s2 = small.tile([1, 2 * TQ], f32, tag="s2", name=f"s2_{p}")
                    nc.vector.tensor_copy(s2[:], yAB[64:65, :])
                    s2_tiles[p] = s2
                    nc.vector.tensor_copy(yhat_sb[0:64, p, :], yA[0:64, :])
                    nc.vector.tensor_copy(yhat_sb[64:128, p, :], yB[0:64, :])

            def s_chunk(p, c, k_t, q_col, pe_tiles, split_exp=False):
                tt, off = c // 4, (c % 4) * 128
                sp = spool.tile([128, 2 * TQ], f32, tag="s", name=f"s_{p}_{c}")
                nc.tensor.matmul(sp[:, 0:TQ], k_t[0:64, tt, off:off + 128],
                                 q_sb[0:64, q_col, :], start=True, stop=True)
                nc.tensor.matmul(sp[:, TQ:2 * TQ], k_t[64:128, tt, off:off + 128],
                                 q_sb[64:128, q_col, :], start=True, stop=True)
                pe_t = pexp.tile([128, 2 * TQ], bf16, tag="pe", name=f"pe_{p}_{c}")
                if split_exp:
                    # y(c,A) can start while the B half is still on ScalarE
                    nc.scalar.activation(pe_t[:, 0:TQ], sp[:, 0:TQ], Exp, scale=0.125)
                    nc.scalar.activation(pe_t[:, TQ:2 * TQ], sp[:, TQ:2 * TQ],
                                         Exp, scale=0.125)
                else:
                    nc.scalar.activation(pe_t[:], sp[:], Exp, scale=0.125)
                pe_tiles[c] = pe_t

            def make_emit_y(p, yA, yB, pe_tiles):
                def emit_y(c):
                    pe_t = pe_tiles.pop(c)
                    vflat = v_sb[:, c].rearrange("p r f -> p (r f)")
                    nc.tensor.matmul(yA[:], vflat[:, p * 144:p * 144 + 128],
                                     pe_t[:, 0:TQ], start=(c == 0), stop=(c == NKT - 1))
                    nc.tensor.matmul(yB[:], vflat[:, p * 144 + 72:p * 144 + 200],
                                     pe_t[:, TQ:2 * TQ],
                                     start=(c == 0), stop=(c == NKT - 1))
                return emit_y


            def emit_v(ci, dh):
                v_ps = mmps.tile([128, TQ], f32, tag="mm", name=f"vps_{ci}_{dh}")
                for cc in range(CC):
                    nc.tensor.matmul(v_ps[:], xt_sb[:, cc, ci * 128:(ci + 1) * 128],
                                     wv_sb[:, dh, cc, :],
                                     start=(cc == 0), stop=(cc == CC - 1))
                nc.vector.tensor_copy(
                    v_sb[:, ci, 4 * dh:4 * dh + 4, :]
                    .rearrange("p r (h f) -> p r h f", h=2)[:, :, :, 0:64],
                    v_ps.rearrange("p (r h f) -> p r h f", r=4, h=2),
                )

            # ---- phase 1b: stream K0/K1 + the dh0 half of V per token chunk,
            # processing pair 0's attention chunks inline — ScalarE's exp
            # stream hides under the V matmuls. ----
            kt0 = kpool.tile([128, 4, TQ], bf16, tag="kp", name="k_0")
            kt1 = kpool.tile([128, 4, TQ], bf16, tag="kp", name="k_1")
            yAB0 = ypool.tile([128, 2 * TQ], f32, tag="y", name="yAB_0")
            pe0 = {}
            emit_y0 = make_emit_y(0, yAB0[:, 0:TQ], yAB0[:, TQ:2 * TQ], pe0)
            for tt in range(4):
                for j, (wk_t, k_t) in enumerate(((wk0, kt0), (wk1, kt1))):
                    k_ps = mmps.tile([128, TQ], f32, tag="mm", name=f"kps_{j}_{tt}")
                    for cc in range(CC):
                        nc.tensor.matmul(k_ps[:], wk_t[:, cc, :],
                                         xt_sb[:, cc, tt * TQ:(tt + 1) * TQ],
                                         start=(cc == 0), stop=(cc == CC - 1))
                    nc.vector.tensor_scalar_add(k_t[:, tt, :], k_ps[:],
                                                ba_sb[:, 8 + j:9 + j])
                for ci in range(4 * tt, 4 * tt + 4):
                    emit_v(ci, 0)
                    s_chunk(0, ci, kt0, 0, pe0)
                    if ci >= 2:
                        emit_y0(ci - 2)
            emit_y0(NKT - 2)
            emit_y0(NKT - 1)
            emit_drain(0, yAB0)
            ktiles = {1: kt1}

            # proj partial accumulator (slot shared with wq_all, whose
            # lifetime ends after the Q phase)
            opart_sb = big.tile([128, CC, TQ], f32, tag="scratch16")

            # V-dh1 drip: pairs 4-7's V half is computed during the exp-paced
            # pairs 1-3 (one transient psum block per kn-ring window; pair 1
            # fits two). {pair: {c_slot: v_chunk}}
            _VDH1_SCHED = {
                1: {1: 0, 3: 1, 5: 2, 7: 3, 9: 4, 11: 5, 13: 6, 15: 7},
                2: {3: 8, 7: 9, 11: 10, 15: 11},
                3: {3: 12, 7: 13, 11: 14, 15: 15},
            }

            # proj-partial drip schedule: pair -> {c_slot: [(tile, cc_lo, single)]}
            # 3 groups per pair at c={5,9,13} for pairs 2-6 (inside the kn-drip
            # ring pattern), remainder in pair 7 which has no K drip.
            _PROJ_SCHED = {
                2: {5: [(0, 0, False)], 9: [(1, 0, False)], 13: [(2, 0, False)]},
                3: {5: [(3, 0, False)], 9: [(4, 0, False)], 13: [(5, 0, False)]},
                4: {5: [(6, 0, False)], 9: [(7, 0, False)], 13: [(0, 2, False)]},
                5: {5: [(1, 2, False)], 9: [(2, 2, False)], 13: [(3, 2, False)]},
                6: {5: [(4, 2, False)], 9: [(5, 2, False)], 13: [(6, 2, False)]},
                7: {3: [(7, 2, False)], 4: [(0, 4, False)], 5: [(1, 4, False)],
                    6: [(2, 4, False)], 7: [(3, 4, False)], 8: [(4, 4, False)],
                    9: [(5, 4, False)], 10: [(6, 4, False)], 11: [(7, 4, False)],
                    12: [(0, 6, True), (1, 6, True)],
                    13: [(2, 6, True), (3, 6, True)],
                    14: [(4, 6, True), (5, 6, True)],
                    15: [(6, 6, True), (7, 6, True)]},
            }

            # ---- pairs 1-7 ----
            # Pipeline per pair: per kt-chunk c emit S(c) -> exp(c) -> y(c-1),
            # with the next pair's K-chunk matmuls drip-fed 2 per chunk so the
            # PE always has exp-independent work while ScalarE runs.
            for p in range(1, NPAIR):
                kt_next = p + 1 if p + 1 < NPAIR else None
                knext_state = {}

                def emit_knext(ci, p=p, kt_next=kt_next, st=None):
                    # two accumulation matmuls of k(p+1) per kt chunk index ci
                    if kt_next is None:
                        return
                    st = knext_state
                    tt, ai = ci // 4, (ci % 4) * 2
                    if ai == 0:
                        st["wk"] = wst.tile([128, CC, 128], bf16, tag="w",
                                            name=f"wkn_{kt_next}_{tt}") if tt == 0 else st["wk"]
                        if tt == 0:
                            nc.sync.dma_start(st["wk"][:], wqk[8 + kt_next])
                            st["kt"] = kpool.tile([128, 4, TQ], bf16, tag="kp",
                                                  name=f"k_{kt_next}")
                        st["ps"] = mmps.tile([128, TQ], f32, tag="mm",
                                             name=f"kn_{kt_next}_{tt}")
                    for cc in (ai, ai + 1):
                        nc.tensor.matmul(st["ps"][:], st["wk"][:, cc, :],
                                         xt_sb[:, cc, tt * TQ:(tt + 1) * TQ],
                                         start=(cc == 0), stop=(cc == CC - 1))
                    if ai == 6:
                        nc.vector.tensor_scalar_add(st["kt"][:, tt, :], st["ps"][:],
                                                    ba_sb[:, 8 + kt_next:9 + kt_next])
                        if tt == 3:
                            ktiles[kt_next] = st["kt"]

                k_t = ktiles.pop(p)
                yAB = ypool.tile([128, 2 * TQ], f32, tag="y", name=f"yAB_{p}")
                pe_tiles = {}
                emit_y = make_emit_y(p, yAB[:, 0:TQ], yAB[:, TQ:2 * TQ], pe_tiles)

                # proj partials spread over pairs 2-7, sized so every pair
                # stays under the exp pace (~170ns/chunk of free PE): 2-cc
                # groups accumulated into opart as the yt columns become
                # available (yt[cc] needs norm(cc), done at pair cc+1 c==2).
                # PROJ_SCHED[p] = [(c_slot, tile, cc_lo or cc6-single)].
                def emit_projpart(c, p=p):
                    sched = _PROJ_SCHED.get(p)
                    if not sched or c not in sched:
                        return
                    for gi, (i, cc_lo, single) in enumerate(sched[c]):
                        tt, oh = i // 2, i % 2
                        pp = mmps.tile([128, TQ], f32, tag="mm",
                                       name=f"pp_{p}_{c}_{gi}")
                        nc.tensor.matmul(pp[:],
                                         yt_sb[:, cc_lo, tt * 128:(tt + 1) * 128],
                                         wp_sb[:, cc_lo, oh * TQ:(oh + 1) * TQ],
                                         start=True, stop=single)
                        if not single:
                            nc.tensor.matmul(
                                pp[:], yt_sb[:, cc_lo + 1, tt * 128:(tt + 1) * 128],
                                wp_sb[:, cc_lo + 1, oh * TQ:(oh + 1) * TQ],
                                start=False, stop=True)
                        if cc_lo == 0:
                            nc.vector.tensor_copy(opart_sb[:, i, :], pp[:])
                        else:
                            nc.vector.tensor_add(opart_sb[:, i, :], opart_sb[:, i, :],
                                                 pp[:])

                for c in range(NKT):
                    s_chunk(p, c, k_t, p, pe_tiles,
                            split_exp=(p == NPAIR - 1 and c == NKT - 1))
                    if c >= 2:
                        emit_y(c - 2)
                    emit_knext(c)
                    vci = _VDH1_SCHED.get(p, {}).get(c)
                    if vci is not None:
                        emit_v(vci, 1)
                    emit_projpart(c)
                    if c == (5 if p == NPAIR - 1 else 3):
                        # past the boundary drain copies so the bc matmuls
                        # never reach the PE before the vector reciprocal
                        # chain has drained; pair 7 (no K drip to absorb
                        # stalls, norm output unused until c>=12) goes later
                        emit_norm(p - 1)
                emit_y(NKT - 2)
                emit_y(NKT - 1)
                emit_drain(p, yAB)

            # ---- output projection: last contraction chunk + stashed partials.
            # Both oh-halves of a token row go into one (now idle) spool tile,
            # so the tail is 4 wide adds and 4 full-row DMAs. ----
            for tt in range(4):
                o_ps = spool.tile([128, 2 * TQ], f32, tag="s", name=f"ops_{tt}")
                for oh in range(2):
                    nc.tensor.matmul(o_ps[:, oh * TQ:(oh + 1) * TQ],
                                     yt_sb[:, CC - 1, tt * 128:(tt + 1) * 128],
                                     wp_sb[:, CC - 1, oh * TQ:(oh + 1) * TQ],
                                     start=True, stop=True)
                o_sb = small.tile([128, 2 * TQ], bf16, tag=f"osb{tt}", name=f"osb_{tt}",
                                  bufs=1)
                nc.vector.tensor_add(
                    o_sb[:], o_ps[:],
                    opart_sb[:, 2 * tt:2 * tt + 2, :].rearrange("p i t -> p (i t)"))
                dma_eng = (nc.sync, nc.scalar, nc.gpsimd)[tt % 3]
                dma_eng.dma_start(out[tt * 128:(tt + 1) * 128, :], o_sb[:])
    nc.compile()
    return nc


def _get_nc():
    if "nc" not in _CACHE:
        _CACHE["nc"] = _build_nc()
    return _CACHE["nc"]


def _in_maps(x, W_attn, b_attn, W_proj, b_proj):
    import ml_dtypes
    bf = ml_dtypes.bfloat16
    x = np.asarray(x, np.float32).reshape(B, T, C)
    W_attn = np.asarray(W_attn, np.float32)
    b_attn = np.asarray(b_attn, np.float32)
    W_proj = np.asarray(W_proj, np.float32)

    xt_all = [x[b_].T.astype(bf) for b_ in range(B)]
    # jc-major contiguous q/k weight blocks: wqk[jc, p, cc, j]
    wqk = np.ascontiguousarray(
        W_attn[:, :2 * C].reshape(CC, 128, 16, 128).transpose(2, 1, 0, 3)
    ).astype(bf)
    # wv[p, dh, cc, t] = W_v[cc*128+p, dh*512+t] — partition-major halves
    wv = np.ascontiguousarray(
        W_attn[:, 2 * C:].reshape(CC, 128, 2, TQ).transpose(1, 2, 0, 3)).astype(bf)
    # wp[p, cc, d] = W_proj[cc*128+p, d]
    wp = np.ascontiguousarray(
        W_proj.reshape(CC, 128, C).transpose(1, 0, 2)).astype(bf)
    # ba pre-shuffled to [128, 16] so the DMA is 128 contiguous 64B rows
    ba = np.ascontiguousarray(b_attn[:2 * C].reshape(16, 128).T.astype(np.float32))

    maps = []
    for i in range(8):
        b_, r = i // 4, i % 4
        xt_b = xt_all[b_]
        if r:
            # roll so this core's query block is token chunk 0 (attention is
            # permutation-invariant over keys)
            xt_b = np.concatenate([xt_b[:, r * TQ:], xt_b[:, :r * TQ]], axis=1)
        # xt[p, cc, t] = xt_b[cc*128+p, t] — partition-major
        xt_b = np.ascontiguousarray(xt_b.reshape(CC, 128, T).transpose(1, 0, 2))
        maps.append({"xt": xt_b, "wqk": wqk, "wv_in": wv, "ba": ba, "wp": wp})
    return maps


def run(x, W_attn, b_attn, W_proj, b_proj, trace=False):
    from concourse.bass_utils import run_bass_kernel_spmd
    nc = _get_nc()
    maps = _in_maps(x, W_attn, b_attn, W_proj, b_proj)
    res = run_bass_kernel_spmd(nc, maps, list(range(8)), trace=trace)
    out = np.empty((B, T, C), np.float32)
    for i in range(8):
        b_, r = i // 4, i % 4
        out[b_, r * TQ:(r + 1) * TQ, :] = res.results[i]["out"].astype(np.float32)
    # v-bias and proj-bias fold: softmax rows sum to 1, so
    # P @ (V + 1 b_v^T) = P @ V + b_v  ->  out += b_v @ W_proj + b_proj  (exact)
    b_attn = np.asarray(b_attn, np.float32)
    b_proj = np.asarray(b_proj, np.float32)
    if b_attn[2 * C:].any() or b_proj.any():
        out += (b_attn[2 * C:] @ np.asarray(W_proj, np.float32) + b_proj).astype(np.float32)
    return out, res


def kernel(x, W_attn, b_attn, W_proj, b_proj):
    out, _ = run(x, W_attn, b_attn, W_proj, b_proj, trace=False)
    return out


# revision 37
# speedup vs baseline: 1.0440x; 1.0005x over previous
"""Trainium2 Bass kernel for nn_AudioSelfAttention (B=2, T=2048, C=1024, H=16).

Sharding: sequence-parallel over the 8 NeuronCores. Core i handles batch
i//4 and query-token slice (i%4)*512. Each core computes K/V for its full
batch locally (redundant within the 4-core batch group — measured collective
cost here, ~76us floor + ~47us/MB, makes the AllGather alternative slower
than recomputation), computes attention for its own 512 query rows over all
16 heads, and the output projection. No collectives; the full output is
assembled on the host from the 8 row-shards.

The per-core xt input is ROLLED on the host so the core's own query block is
token-chunk 0 (attention is permutation-invariant over keys): q is computed
straight from xt chunk 0 and no separate xq input is needed.

Engine balance: ScalarE's exp stream (~1.25us per kt-chunk) is the pair-phase
floor, so pair 0's attention chunks are processed INLINE with the K0/K1/V
streaming phase (PE-heavy, exp fully hidden), and pairs 1-7 each drip-feed
the NEXT pair's K projection (2 matmuls per chunk) plus a spread schedule of
proj-partial groups, sized so every pair stays just under the exp pace.
xt is DMA'd in token chunks (chunk 0 feature-chunk-wise) so the Q phase
starts after ~130KB. The sync DMA queue carries only small weight blocks — a
descriptor-ring-full stall on the sync engine blocks tile-pool boundaries
and thus the PE — and all inputs are pre-shuffled on the host to
partition-major layouts so every DMA trigger generates only ~128 descriptors.

Compute dtype: bf16 matmul operands (fp32 matmul is 4 cycles/row on TRN2's
PE vs 1 for bf16), fp32 PSUM accumulation and softmax statistics. Output is
stored bf16 (quantization ~0.4% rel, well within budget) and upcast on host.

Layouts: activations live in transposed [feature, token] space so every
matmul contracts along partitions. Attention scores are computed as
S^T = K^T-chunks x Q^T (kt on partitions), softmax-exp runs on ScalarE over
4-bank PSUM spans, and the softmax denominator falls out of the y-matmul by
augmenting V with a ones-column (M=65). Per-pair normalization uses two tiny
half-broadcast matmuls (ones-row stationary) to spread 1/sum across
partitions. The v-part and proj biases are folded in exactly on the host
(softmax rows sum to 1, so they reduce to a constant row added to the
output); q/k biases are applied on-device in the PSUM->SBUF copies.
All proj contributions except the last feature chunk are accumulated during
the final two head-pairs, so the post-attention tail is just the last-pair
normalization, 8 single matmuls, and the (bf16) output DMA.
"""
import numpy as np

_CACHE = {}

B, T, C, H, D = 2, 2048, 1024, 16, 64
TQ = T * B // 8          # 512 query tokens per core
CC = C // 128            # 8 contraction chunks
NPAIR = H // 2           # 8 head pairs
NKT = T // 128           # 16 kt chunks


def _build_nc():
    import concourse.bacc as bacc
    import concourse.tile as tile
    import concourse.mybir as mybir

    f32 = mybir.dt.float32
    bf16 = mybir.dt.bfloat16
    Exp = mybir.ActivationFunctionType.Exp

    nc = bacc.Bacc(None, num_devices=8)
    # All inputs are pre-shuffled on the host into partition-major layouts so
    # every DMA is ~128 contiguous per-partition segments (cheap descriptor
    # generation — a (c p)-rearranged DMA costs 1024 descriptors and 8-28us
    # of trigger time on the issuing engine).
    # xt[p, cc, t]: rolled so this core's query block is token chunk 0
    xt = nc.declare_dram_parameter("xt", [128, CC, T], bf16, isOutput=False)
    # wqk[jc, p, cc, j] = W_attn[cc*128+p, jc*128+j]  (jc 0..7 = q, 8..15 = k)
    wqk = nc.declare_dram_parameter("wqk", [16, 128, CC, 128], bf16, isOutput=False)
    wv_in = nc.declare_dram_parameter("wv_in", [128, 2, CC, TQ], bf16, isOutput=False)
    ba = nc.declare_dram_parameter("ba", [128, 16], f32, isOutput=False)
    wp = nc.declare_dram_parameter("wp", [128, CC, C], bf16, isOutput=False)
    out = nc.declare_dram_parameter("out", [TQ, C], bf16, isOutput=True)

    with tile.TileContext(nc) as tc:
        with (
            tc.tile_pool(name="big", bufs=1) as big,
            tc.tile_pool(name="wst", bufs=3) as wst,
            tc.tile_pool(name="kpool", bufs=4) as kpool,
            tc.tile_pool(name="pexp", bufs=6) as pexp,
            tc.tile_pool(name="small", bufs=2) as small,
            tc.tile_pool(name="mmps", bufs=2, space="PSUM") as mmps,
            tc.tile_pool(name="spool", bufs=2, space="PSUM") as spool,
            tc.tile_pool(name="ypool", bufs=1, space="PSUM") as ypool,
        ):
            # ---- DMA schedule.
            # scalar queue: xt chunk 0 (cc-wise, so the Q phase starts after
            #   ~130KB), wv in dh halves, wp.
            # sync queue: wq blocks + ba, wk01, xt token chunks 1-3 (cc-wise
            #   contiguous tails), then per-pair wk drip later. ----
            xt_sb = big.tile([128, CC, T], bf16)
            # tiny head slice first: the PE warm-up gates only on these 16KB
            nc.scalar.dma_start(xt_sb[:, 0, 0:64], xt[:, 0, 0:64])
            nc.scalar.dma_start(xt_sb[:, 0, 64:TQ], xt[:, 0, 64:TQ])
            for cc in range(1, CC):
                nc.scalar.dma_start(xt_sb[:, cc, 0:TQ], xt[:, cc, 0:TQ])

            # wq shares its slot with the proj partial accumulator (disjoint lifetimes)
            wq_all = big.tile([128, CC, CC, 128], bf16, tag="scratch16")
            for jc in range(2):
                nc.sync.dma_start(wq_all[:, jc, :, :], wqk[jc])
            ba_sb = big.tile([128, 16], f32)
            nc.sync.dma_start(ba_sb[:], ba[:])
            # all wq blocks before wk01: the Q phase consumes wq pairs every
            # ~3.5us while wk isn't needed until K01(tt0) after Q ends
            for jc in range(2, CC):
                nc.sync.dma_start(wq_all[:, jc, :, :], wqk[jc])
            wk0 = wst.tile([128, CC, 128], bf16, tag="w", name="wk_0")
            nc.sync.dma_start(wk0[:], wqk[8])
            wk1 = wst.tile([128, CC, 128], bf16, tag="w", name="wk_1")
            nc.sync.dma_start(wk1[:], wqk[9])

            # bulk tails ride the scalar queue — the sync queue stays lean so
            # its engine never hits a ring-full stall (tile-pool boundaries
            # ride the sync engine and a stalled trigger there blocks the PE)
            wv_sb = big.tile([128, 2, CC, TQ], bf16)
            nc.scalar.dma_start(wv_sb[:, 0], wv_in[:, 0])
            for cc in range(CC):
                nc.scalar.dma_start(xt_sb[:, cc, TQ:T], xt[:, cc, TQ:T])
            # dh1 half of wv is consumed only by the V drip in pairs 1-3
            nc.scalar.dma_start(wv_sb[:, 1], wv_in[:, 1])
            wp_sb = big.tile([128, CC, C], bf16)
            nc.scalar.dma_start(wp_sb[:], wp[:])

            sel_sb = big.tile([1, 64], bf16)
            nc.vector.memset(sel_sb[:], 1.0)
            q_sb = big.tile([128, CC, TQ], bf16)
            # v pair blocks padded to 144 cols (288B, 16B-aligned for both
            # head slices): [vA(64) | 1 | pad(7) | vB(64) | 1 | pad(7)].
            # One extra pad block lets M=128 stationary reads over-run (junk
            # cols only land in unread PSUM partitions 65..127).
            v_sb = big.tile([128, NKT, NPAIR + 1, 144], bf16)
            nc.vector.memset(
                v_sb.rearrange("p t r (h f) -> p t r h f", h=2)[:, :, :, :, 64:65], 1.0
            )
            yt_sb = big.tile([128, CC, TQ], bf16)
            yhat_sb = big.tile([128, CC, TQ], bf16)

            # ---- PE warm-up: the PE needs ~3us of continuous work to reach
            # the 2.4GHz p-state; burn the DMA lead-in on dummy matmuls over
            # the first xt chunk (output discarded) ----
            warm_ps = mmps.tile([128, TQ], f32, tag="mm", name="warm_ps")
            for w in range(40):
                nc.tensor.matmul(warm_ps[0:64, 0:64], xt_sb[:, 0, 0:64],
                                 xt_sb[:, 0, 0:64], start=True, stop=True)

            # ---- Q phase: q^T[j, tq] = W_q^T @ xt[:, chunk0], cc-outer over
            # jc pairs so the first matmul needs only wq[0,1] + xt chunk0/cc0.
            # PSUM alternates mmps/ypool so groups don't stall on rotation. ----
            for jcg in range(CC // 2):
                jA, jB = 2 * jcg, 2 * jcg + 1
                if jcg % 2 == 0:
                    qA = mmps.tile([128, TQ], f32, tag="mm", name=f"qps_{jA}")
                    qB = mmps.tile([128, TQ], f32, tag="mm", name=f"qps_{jB}")
                else:
                    qAB = ypool.tile([128, 2 * TQ], f32, tag="y", name=f"qps_{jA}{jB}")
                    qA, qB = qAB[:, 0:TQ], qAB[:, TQ:2 * TQ]
                for cc in range(CC):
                    nc.tensor.matmul(qA[:], wq_all[:, jA, cc, :], xt_sb[:, cc, 0:TQ],
                                     start=(cc == 0), stop=(cc == CC - 1))
                    nc.tensor.matmul(qB[:], wq_all[:, jB, cc, :], xt_sb[:, cc, 0:TQ],
                                     start=(cc == 0), stop=(cc == CC - 1))
                nc.vector.tensor_scalar_add(q_sb[:, jA, :], qA[:], ba_sb[:, jA:jA + 1])
                nc.vector.tensor_scalar_add(q_sb[:, jB, :], qB[:], ba_sb[:, jB:jB + 1])

            # deferred per-pair normalization: emitted 3 chunks into the NEXT
            # pair so the sums->reciprocal chain never stalls the in-order PE
            # at a pair boundary
            s2_tiles = {}

            def emit_norm(p):
                s2 = s2_tiles.pop(p)
                r2 = small.tile([1, 2 * TQ], f32, tag="r2", name=f"r2_{p}", bufs=1)
                nc.vector.reciprocal_approx_fast(r2[:], s2[:])
                rb2 = small.tile([1, 2 * TQ], bf16, tag="rb2", name=f"rb2_{p}", bufs=1)
                nc.vector.tensor_copy(rb2[:], r2[:])
                bc = mmps.tile([128, TQ], f32, tag="mm", name=f"bc_{p}")
                nc.tensor.matmul(bc[0:64, :], sel_sb[:], rb2[:, 0:TQ], start=True,
                                 stop=True)
                nc.tensor.matmul(bc[64:128, :], sel_sb[:], rb2[:, TQ:2 * TQ],
                                 start=True, stop=True)
                nc.vector.tensor_mul(yt_sb[0:64, p, :], yhat_sb[0:64, p, :], bc[0:64, :])
                nc.vector.tensor_mul(yt_sb[64:128, p, :], yhat_sb[64:128, p, :],
                                     bc[64:128, :])

            def emit_drain(p, yAB):
                yA, yB = yAB[:, 0:TQ], yAB[:, TQ:2 * TQ]
                if p == NPAIR - 1:
                    # last pair: sums extracted on the (idle) ScalarE so the
                    # yhat copies run concurrently on vector
                    s2 = small.tile([1, 2 * TQ], f32, tag="s2", name=f"s2_{p}")
                    nc.scalar.activation(s2[:], yAB[64:65, :],
                                         mybir.ActivationFunctionType.Copy)
                    nc.vector.tensor_copy(yhat_sb[0:64, p, :], yA[0:64, :])
                    nc.vector.tensor_copy(yhat_sb[64:128, p, :], yB[0:64, :])
                    r2 = small.tile([1, 2 * TQ], f32, tag="r2", name=f"r2_{p}", bufs=1)
                    nc.vector.reciprocal_approx_fast(r2[:], s2[:])
                    rb2 = small.tile([1, 2 * TQ], bf16, tag="rb2", name=f"rb2_{p}",
                                     bufs=1)
                    nc.vector.tensor_copy(rb2[:], r2[:])
                    # keep the PE hot through the norm-chain wait (discarded
                    # matmuls) so the bc + final proj matmuls run at 2.4GHz
                    # instead of the 1.2GHz post-idle p-state
                    warm2 = mmps.tile([128, TQ], f32, tag="mm", name="warm_drain")
                    for w in range(14):
                        nc.tensor.matmul(warm2[:], xt_sb[:, 0, 0:128],
                                         xt_sb[:, 0, 0:TQ], start=True, stop=True)
                    bc = mmps.tile([128, TQ], f32, tag="mm", name=f"bc_{p}")
                    nc.tensor.matmul(bc[0:64, :], sel_sb[:], rb2[:, 0:TQ], start=True,
                                     stop=True)
                    nc.tensor.matmul(bc[64:128, :], sel_sb[:], rb2[:, TQ:2 * TQ],
                                     start=True, stop=True)
                    for w in range(6):
                        nc.tensor.matmul(warm2[:], xt_sb[:, 0, 0:128],
                                         xt_sb[:, 0, 0:TQ], start=True, stop=True)
                    nc.vector.tensor_mul(yt_sb[0:64, p, :], yhat_sb[0:64, p, :],
                                         bc[0:64, :])
                    nc.vector.tensor_mul(yt_sb[64:128, p, :], yhat_sb[64:128, p, :],
                                         bc[64:128, :])
                else:
                    # stash unnormalized y + sums, freeing the accumulator
                    s2 = small.tile([1, 2 * TQ], f32, tag="s2", name=f"s2_{p}")
                    nc.vector.tensor_copy(s2[:], yAB[64:65, :])
                    s2_tiles[p] = s2
                    nc.vector.tensor_copy(yhat_sb[0:64, p, :], yA[0:64, :])
                    nc.vector.tensor_copy(yhat_sb[64:128, p, :], yB[0:64, :])

            def s_chunk(p, c, k_t, q_col, pe_tiles, split_exp=False):
                tt, off = c // 4, (c % 4) * 128
                sp = spool.tile([128, 2 * TQ], f32, tag="s", name=f"s_{p}_{c}")
                nc.tensor.matmul(sp[:, 0:TQ], k_t[0:64, tt, off:off + 128],
                                 q_sb[0:64, q_col, :], start=True, stop=True)
                nc.tensor.matmul(sp[:, TQ:2 * TQ], k_t[64:128, tt, off:off + 128],
                                 q_sb[64:128, q_col, :], start=True, stop=True)
                pe_t = pexp.tile([128, 2 * TQ], bf16, tag="pe", name=f"pe_{p}_{c}")
                if split_exp:
                    # y(c,A) can start while the B half is still on ScalarE
                    nc.scalar.activation(pe_t[:, 0:TQ], sp[:, 0:TQ], Exp, scale=0.125)
                    nc.scalar.activation(pe_t[:, TQ:2 * TQ], sp[:, TQ:2 * TQ],
                                         Exp, scale=0.125)
                else:
                    nc.scalar.activation(pe_t[:], sp[:], Exp, scale=0.125)
                pe_tiles[c] = pe_t

            def make_emit_y(p, yA, yB, pe_tiles):
                def emit_y(c):
                    pe_t = pe_tiles.pop(c)
                    vflat = v_sb[:, c].rearrange("p r f -> p (r f)")
                    nc.tensor.matmul(yA[:], vflat[:, p * 144:p * 144 + 128],
                                     pe_t[:, 0:TQ], start=(c == 0), stop=(c == NKT - 1))
                    nc.tensor.matmul(yB[:], vflat[:, p * 144 + 72:p * 144 + 200],
                                     pe_t[:, TQ:2 * TQ],
                                     start=(c == 0), stop=(c == NKT - 1))
                return emit_y


            def emit_v(ci, dh):
                v_ps = mmps.tile([128, TQ], f32, tag="mm", name=f"vps_{ci}_{dh}")
                for cc in range(CC):
                    nc.tensor.matmul(v_ps[:], xt_sb[:, cc, ci * 128:(ci + 1) * 128],
                                     wv_sb[:, dh, cc, :],
                                     start=(cc == 0), stop=(cc == CC - 1))
                nc.vector.tensor_copy(
                    v_sb[:, ci, 4 * dh:4 * dh + 4, :]
                    .rearrange("p r (h f) -> p r h f", h=2)[:, :, :, 0:64],
                    v_ps.rearrange("p (r h f) -> p r h f", r=4, h=2),
                )

            # ---- phase 1b: stream K0/K1 + the dh0 half of V per token chunk,
            # processing pair 0's attention chunks inline — ScalarE's exp
            # stream hides under the V matmuls. ----
            kt0 = kpool.tile([128, 4, TQ], bf16, tag="kp", name="k_0")
            kt1 = kpool.tile([128, 4, TQ], bf16, tag="kp", name="k_1")
            yAB0 = ypool.tile([128, 2 * TQ], f32, tag="y", name="yAB_0")
            pe0 = {}
            emit_y0 = make_emit_y(0, yAB0[:, 0:TQ], yAB0[:, TQ:2 * TQ], pe0)
            for tt in range(4):
                for j, (wk_t, k_t) in enumerate(((wk0, kt0), (wk1, kt1))):
                    k_ps = mmps.tile([128, TQ], f32, tag="mm", name=f"kps_{j}_{tt}")
                    for cc in range(CC):
                        nc.tensor.matmul(k_ps[:], wk_t[:, cc, :],
                                         xt_sb[:, cc, tt * TQ:(tt + 1) * TQ],
                                         start=(cc == 0), stop=(cc == CC - 1))
                    nc.vector.tensor_scalar_add(k_t[:, tt, :], k_ps[:],
                                                ba_sb[:, 8 + j:9 + j])
                for ci in range(4 * tt, 4 * tt + 4):
                    emit_v(ci, 0)
                    s_chunk(0, ci, kt0, 0, pe0)
                    if ci >= 2:
                        emit_y0(ci - 2)
            emit_y0(NKT - 2)
            emit_y0(NKT - 1)
            emit_drain(0, yAB0)
            ktiles = {1: kt1}

            # proj partial accumulator (slot shared with wq_all, whose
            # lifetime ends after the Q phase)
            opart_sb = big.tile([128, CC, TQ], f32, tag="scratch16")

            # V-dh1 drip: pairs 4-7's V half is computed during the exp-paced
            # pairs 1-3 (one transient psum block per kn-ring window; pair 1
            # fits two). {pair: {c_slot: v_chunk}}
            _VDH1_SCHED = {
                1: {1: 0, 3: 1, 5: 2, 7: 3, 9: 4, 11: 5, 13: 6, 15: 7},
                2: {3: 8, 7: 9, 11: 10, 15: 11},
                3: {3: 12, 7: 13, 11: 14, 15: 15},
            }

            # proj-partial drip schedule: pair -> {c_slot: [(tile, cc_lo, single)]}
            # 3 groups per pair at c={5,9,13} for pairs 2-6 (inside the kn-drip
            # ring pattern), remainder in pair 7 which has no K drip.
            _PROJ_SCHED = {
                2: {5: [(0, 0, False)], 9: [(1, 0, False)], 13: [(2, 0, False)]},
                3: {5: [(3, 0, False)], 9: [(4, 0, False)], 13: [(5, 0, False)]},
                4: {5: [(6, 0, False)], 9: [(7, 0, False)], 13: [(0, 2, False)]},
                5: {5: [(1, 2, False)], 9: [(2, 2, False)], 13: [(3, 2, False)]},
                6: {5: [(4, 2, False)], 9: [(5, 2, False)], 13: [(6, 2, False)]},
                7: {3: [(7, 2, False)], 4: [(0, 4, False)], 5: [(1, 4, False)],
                    6: [(2, 4, False)], 7: [(3, 4, False)], 8: [(4, 4, False)],
                    9: [(5, 4, False)], 10: [(6, 4, False)], 11: [(7, 4, False)],
                    12: [(0, 6, True), (1, 6, True)],
                    13: [(2, 6, True), (3, 6, True)],
                    14: [(4, 6, True), (5, 6, True)],
                    15: [(6, 6, True), (7, 6, True)]},
            }

            # ---- pairs 1-7 ----
            # Pipeline per pair: per kt-chunk c emit S(c) -> exp(c) -> y(c-1),
            # with the next pair's K-chunk matmuls drip-fed 2 per chunk so the
            # PE always has exp-independent work while ScalarE runs.
            for p in range(1, NPAIR):
                kt_next = p + 1 if p + 1 < NPAIR else None
                knext_state = {}

                def emit_knext(ci, p=p, kt_next=kt_next, st=None):
                    # two accumulation matmuls of k(p+1) per kt chunk index ci
                    if kt_next is None:
                        return
                    st = knext_state
                    tt, ai = ci // 4, (ci % 4) * 2
                    if ai == 0:
                        st["wk"] = wst.tile([128, CC, 128], bf16, tag="w",
                                            name=f"wkn_{kt_next}_{tt}") if tt == 0 else st["wk"]
                        if tt == 0:
                            nc.sync.dma_start(st["wk"][:], wqk[8 + kt_next])
                            st["kt"] = kpool.tile([128, 4, TQ], bf16, tag="kp",
                                                  name=f"k_{kt_next}")
                        st["ps"] = mmps.tile([128, TQ], f32, tag="mm",
                                             name=f"kn_{kt_next}_{tt}")
                    for cc in (ai, ai + 1):
                        nc.tensor.matmul(st["ps"][:], st["wk"][:, cc, :],
                                         xt_sb[:, cc, tt * TQ:(tt + 1) * TQ],
                                         start=(cc == 0), stop=(cc == CC - 1))
                    if ai == 6:
                        nc.vector.tensor_scalar_add(st["kt"][:, tt, :], st["ps"][:],
                                                    ba_sb[:, 8 + kt_next:9 + kt_next])
                        if tt == 3:
                            ktiles[kt_next] = st["kt"]

                k_t = ktiles.pop(p)
                yAB = ypool.tile([128, 2 * TQ], f32, tag="y", name=f"yAB_{p}")
                pe_tiles = {}
                emit_y = make_emit_y(p, yAB[:, 0:TQ], yAB[:, TQ:2 * TQ], pe_tiles)

                # proj partials spread over pairs 2-7, sized so every pair
                # stays under the exp pace (~170ns/chunk of free PE): 2-cc
                # groups accumulated into opart as the yt columns become
                # available (yt[cc] needs norm(cc), done at pair cc+1 c==2).
                # PROJ_SCHED[p] = [(c_slot, tile, cc_lo or cc6-single)].
                def emit_projpart(c, p=p):
                    sched = _PROJ_SCHED.get(p)
                    if not sched or c not in sched:
                        return
                    for gi, (i, cc_lo, single) in enumerate(sched[c]):
                        tt, oh = i // 2, i % 2
                        pp = mmps.tile([128, TQ], f32, tag="mm",
                                       name=f"pp_{p}_{c}_{gi}")
                        nc.tensor.matmul(pp[:],
                                         yt_sb[:, cc_lo, tt * 128:(tt + 1) * 128],
                                         wp_sb[:, cc_lo, oh * TQ:(oh + 1) * TQ],
                                         start=True, stop=single)
                        if not single:
                            nc.tensor.matmul(
                                pp[:], yt_sb[:, cc_lo + 1, tt * 128:(tt + 1) * 128],
                                wp_sb[:, cc_lo + 1, oh * TQ:(oh + 1) * TQ],
                                start=False, stop=True)
                        if cc_lo == 0:
                            nc.vector.tensor_copy(opart_sb[:, i, :], pp[:])
                        else:
                            nc.vector.tensor_add(opart_sb[:, i, :], opart_sb[:, i, :],
                                                 pp[:])

                for c in range(NKT):
                    s_chunk(p, c, k_t, p, pe_tiles,
                            split_exp=(p == NPAIR - 1 and c == NKT - 1))
                    if c >= 2:
                        emit_y(c - 2)
                    emit_knext(c)
                    vci = _VDH1_SCHED.get(p, {}).get(c)
                    if vci is not None:
                        emit_v(vci, 1)
                    emit_projpart(c)
                    if c == (5 if p == NPAIR - 1 else 3):
                        # past the boundary drain copies so the bc matmuls
                        # never reach the PE before the vector reciprocal
                        # chain has drained; pair 7 (no K drip to absorb
                        # stalls, norm output unused until c>=12) goes later
                        emit_norm(p - 1)
                emit_y(NKT - 2)
                emit_y(NKT - 1)
                emit_drain(p, yAB)

            # ---- output projection: last contraction chunk + stashed partials.
            # Both oh-halves of a token row go into one (now idle) spool tile,
            # so the tail is 4 wide adds and 4 full-row DMAs. ----
            for tt in range(4):
                o_ps = spool.tile([128, 2 * TQ], f32, tag="s", name=f"ops_{tt}")
                for oh in range(2):
                    nc.tensor.matmul(o_ps[:, oh * TQ:(oh + 1) * TQ],
                                     yt_sb[:, CC - 1, tt * 128:(tt + 1) * 128],
                                     wp_sb[:, CC - 1, oh * TQ:(oh + 1) * TQ],
                                     start=True, stop=True)
                o_sb = small.tile([128, 2 * TQ], bf16, tag=f"osb{tt}", name=f"osb_{tt}",
                                  bufs=1)
                nc.vector.tensor_add(
                    o_sb[:], o_ps[:],
                    opart_sb[:, 2 * tt:2 * tt + 2, :].rearrange("p i t -> p (i t)"))
                dma_eng = (nc.sync, nc.scalar, nc.gpsimd)[tt % 3]
                dma_eng.dma_start(out[tt * 128:(tt + 1) * 128, :], o_sb[:])
    nc.compile()
    return nc


def _get_nc():
    if "nc" not in _CACHE:
        _CACHE["nc"] = _build_nc()
    return _CACHE["nc"]


def _in_maps(x, W_attn, b_attn, W_proj, b_proj):
    import ml_dtypes
    bf = ml_dtypes.bfloat16
    x = np.asarray(x, np.float32).reshape(B, T, C)
    W_attn = np.asarray(W_attn, np.float32)
    b_attn = np.asarray(b_attn, np.float32)
    W_proj = np.asarray(W_proj, np.float32)

    xt_all = [x[b_].T.astype(bf) for b_ in range(B)]
    # jc-major contiguous q/k weight blocks: wqk[jc, p, cc, j]
    wqk = np.ascontiguousarray(
        W_attn[:, :2 * C].reshape(CC, 128, 16, 128).transpose(2, 1, 0, 3)
    ).astype(bf)
    # wv[p, dh, cc, t] = W_v[cc*128+p, dh*512+t] — partition-major halves
    wv = np.ascontiguousarray(
        W_attn[:, 2 * C:].reshape(CC, 128, 2, TQ).transpose(1, 2, 0, 3)).astype(bf)
    # wp[p, cc, d] = W_proj[cc*128+p, d]
    wp = np.ascontiguousarray(
        W_proj.reshape(CC, 128, C).transpose(1, 0, 2)).astype(bf)
    # ba pre-shuffled to [128, 16] so the DMA is 128 contiguous 64B rows
    ba = np.ascontiguousarray(b_attn[:2 * C].reshape(16, 128).T.astype(np.float32))

    maps = []
    for i in range(8):
        b_, r = i // 4, i % 4
        xt_b = xt_all[b_]
        if r:
            # roll so this core's query block is token chunk 0 (attention is
            # permutation-invariant over keys)
            xt_b = np.concatenate([xt_b[:, r * TQ:], xt_b[:, :r * TQ]], axis=1)
        # xt[p, cc, t] = xt_b[cc*128+p, t] — partition-major
        xt_b = np.ascontiguousarray(xt_b.reshape(CC, 128, T).transpose(1, 0, 2))
        maps.append({"xt": xt_b, "wqk": wqk, "wv_in": wv, "ba": ba, "wp": wp})
    return maps


def run(x, W_attn, b_attn, W_proj, b_proj, trace=False):
    from concourse.bass_utils import run_bass_kernel_spmd
    nc = _get_nc()
    maps = _in_maps(x, W_attn, b_attn, W_proj, b_proj)
    res = run_bass_kernel_spmd(nc, maps, list(range(8)), trace=trace)
    out = np.empty((B, T, C), np.float32)
    for i in range(8):
        b_, r = i // 4, i % 4
        out[b_, r * TQ:(r + 1) * TQ, :] = res.results[i]["out"].astype(np.float32)
    # v-bias and proj-bias fold: softmax rows sum to 1, so
    # P @ (V + 1 b_v^T) = P @ V + b_v  ->  out += b_v @ W_proj + b_proj  (exact)
    b_attn = np.asarray(b_attn, np.float32)
    b_proj = np.asarray(b_proj, np.float32)
    if b_attn[2 * C:].any() or b_proj.any():
        out += (b_attn[2 * C:] @ np.asarray(W_proj, np.float32) + b_proj).astype(np.float32)
    return out, res


def kernel(x, W_attn, b_attn, W_proj, b_proj):
    out, _ = run(x, W_attn, b_attn, W_proj, b_proj, trace=False)
    return out
